# revision 1
# baseline (speedup 1.0000x reference)
"""CGCNN conv kernel for 8 TRN2 NeuronCores (Bass/Tile).

Strategy (edge-parallel, dst-sharded):
  z @ W = nf[src] @ W[0:64] + nf[dst] @ W[64:128] + ef @ W[128:160]
  - Host precomputes P_src = nf @ [Wi[:64]|Wu[:64]]  (bf16 [N,128], 256B rows)
                    P_dst = nf @ [Wi[64:128]|Wu[64:128]]
  - Edges sorted by (dst//R, src//CH, src): core c owns dst range
    [c*R,(c+1)*R) so the segment-sum is core-local (no [N,F] all-reduce);
    within a core edges are grouped into src-chunks of CH=25000 so gather
    indices fit int16.
  - Pass 1: transposed dma_gather of P rows -> feat-major [128,T] tiles;
    PE adds the edge-feat matmul; DVE ttr assembles x (+sum); ACT
    square-accum (+sumsq); x stored to DRAM bf16.
  - AllReduce [128,2] edge-BN stats.
  - Pass 2: reload x; gate = Sigmoid(s*x+b); softplus via Ln(1+Exp(.))
    (no softplus table on TRN2); msg transposed to row-major on PE;
    dma_scatter_add into per-core agg [R_pad, 64].
  - Phase 3: node-BN stats AllReduce [64,2]; out = softplus(nf + bn(agg))
    computed feat-major; host transposes back.
"""

import math
import sys

import numpy as np

for _p in ("/opt/trn_rl_repo", "/root/.axon_site/_ro/trn_rl_repo"):
    if _p not in sys.path:
        sys.path.append(_p)

import ml_dtypes
from concourse import bacc, bass, mybir
from concourse import tile as ctile
from concourse.bass_utils import run_bass_kernel_spmd
from concourse.masks import make_identity

P = 128
F = 64  # node feature dim; 2F == P
EPS = 1e-5
BF16 = ml_dtypes.bfloat16

Alu = mybir.AluOpType
Act = mybir.ActivationFunctionType
dt = mybir.dt


def _cfg(N, E, FE, T=2048, sub=512, g_batch=6, ncores=8):
    R = N // ncores
    assert R * ncores == N
    nchunk = max(1, math.ceil(N / 25000))
    CH = math.ceil(N / nchunk)
    assert CH + 1 <= 32768 and R + 1 <= 32768
    r_pad = math.ceil((R + 1) / P) * P
    return dict(
        N=N, E=E, FE=FE, T=T, SUB=sub, G=g_batch, NC=ncores,
        R=R, NCHUNK=nchunk, CH=CH, R_PAD=r_pad,
    )


def build_graph(cfg, debug=False):
    NC, T, SUB, FE = cfg["NC"], cfg["T"], cfg["SUB"], cfg["FE"]
    CH, NCHUNK, R_PAD = cfg["CH"], cfg["NCHUNK"], cfg["R_PAD"]
    TPC, ETOT = cfg["TPC"], cfg["ETOT"]
    SEGS = list(cfg["SEGS"])
    nseg = len(SEGS)
    NTILES = NCHUNK * TPC
    NBLK = NTILES // 2
    NGRP = R_PAD // P
    nsub = T // SUB
    inv_e = 1.0 / float(cfg["E"])
    inv_n = 1.0 / float(cfg["N"])

    nc = bacc.Bacc("TRN2", target_bir_lowering=False, debug=False,
                   num_devices=NC)

    psrc = [nc.dram_tensor(f"psrc{c}", [CH + 1, P], dt.bfloat16,
                           kind="ExternalInput") for c in range(NCHUNK)]
    pdst = nc.dram_tensor("pdst", [R_PAD, P], dt.bfloat16, kind="ExternalInput")
    eft = nc.dram_tensor("eft", [FE, ETOT], dt.bfloat16, kind="ExternalInput")
    srcidx = nc.dram_tensor("srcidx", [P, ETOT // 16], dt.int16,
                            kind="ExternalInput")
    dstidx = nc.dram_tensor("dstidx", [P, ETOT // 16], dt.int16,
                            kind="ExternalInput")
    nft = nc.dram_tensor("nft", [F, R_PAD], dt.float32, kind="ExternalInput")
    w3 = nc.dram_tensor("w3", [FE, P], dt.bfloat16, kind="ExternalInput")
    gvec = nc.dram_tensor("gvec", [P, 1], dt.float32, kind="ExternalInput")
    bvec = nc.dram_tensor("bvec", [P, 1], dt.float32, kind="ExternalInput")
    gbn = nc.dram_tensor("gbn", [F, 1], dt.float32, kind="ExternalInput")
    bbn = nc.dram_tensor("bbn", [F, 1], dt.float32, kind="ExternalInput")
    outT = nc.dram_tensor("outT", [F, R_PAD], dt.float32, kind="ExternalOutput")

    xint = nc.dram_tensor("xint", [NBLK, P, T], dt.bfloat16, kind="Internal")
    xupd = nc.dram_tensor("xupd", [NBLK, P, T], dt.bfloat16, kind="Internal")
    aggd = [nc.dram_tensor(f"aggd{r}", [NGRP, P, F], dt.float32,
                           kind="Internal") for r in range(nseg + 1)]
    cc1i = nc.dram_tensor("cc1i", [P, 2], dt.float32, kind="Internal")
    cc1o = nc.dram_tensor("cc1o", [P, 2], dt.float32, kind="Internal",
                          addr_space="Shared")
    cc2i = nc.dram_tensor("cc2i", [F, 2], dt.float32, kind="Internal")
    cc2o = nc.dram_tensor("cc2o", [F, 2], dt.float32, kind="Internal",
                          addr_space="Shared")

    rg = [list(range(NC))]
    if debug:
        dbg_xint = nc.dram_tensor("dbg_xint", [NBLK, P, T], dt.bfloat16,
                                  kind="ExternalOutput")
        dbg_agg = nc.dram_tensor("dbg_agg", [NGRP, P, F], dt.float32,
                                 kind="ExternalOutput")
        dbg_st = nc.dram_tensor("dbg_st", [P, 12], dt.float32,
                                kind="ExternalOutput")

    with ctile.TileContext(nc) as tc:
        with tc.tile_pool(name="const", bufs=1) as cp:
            w3_sb = cp.tile([FE, P], dt.bfloat16)
            nc.sync.dma_start(w3_sb[:], w3.ap())
            identb = cp.tile([P, P], dt.bfloat16)
            make_identity(nc, identb[:])
            identf = cp.tile([F, F], dt.float32)
            make_identity(nc, identf[:])
            identf128 = cp.tile([P, P], dt.float32)
            make_identity(nc, identf128[:])
            gv = cp.tile([P, 1], dt.float32)
            nc.sync.dma_start(gv[:], gvec.ap())
            bv = cp.tile([P, 1], dt.float32)
            nc.sync.dma_start(bv[:], bvec.ap())
            gbn_sb = cp.tile([F, 1], dt.float32)
            nc.sync.dma_start(gbn_sb[:], gbn.ap())
            bbn_sb = cp.tile([F, 1], dt.float32)
            nc.sync.dma_start(bbn_sb[:], bbn.ap())

            sumc = cp.tile([P, NTILES * nsub], dt.float32)
            sqc = cp.tile([P, NTILES * nsub], dt.float32)

            # zero-fill agg accumulator
            zb = cp.tile([P, SUB], dt.float32)
            nc.vector.memset(zb[:], 0.0)
            gper = SUB // F  # groups of [P,F] per zero DMA
            for r in range(nseg + 1):
                for g0 in range(0, NGRP, gper):
                    ng = min(gper, NGRP - g0)
                    nc.sync.dma_start(aggd[r].ap()[g0:g0 + ng, :, :],
                                      zb[:, :ng * F])

            # ---------------- pass 1 ----------------
            with tc.tile_pool(name="p1", bufs=4) as p1, \
                 tc.tile_pool(name="p1i", bufs=6) as p1i, \
                 tc.tile_pool(name="ps1", bufs=4, space="PSUM") as ps1:
                for c in range(NCHUNK):
                    for tl in range(TPC):
                        t = c * TPC + tl
                        col0 = t * (T // 16)
                        sidx = p1i.tile([P, T // 16], dt.int16, tag="sidx")
                        nc.sync.dma_start(sidx[:],
                                          srcidx.ap()[:, col0:col0 + T // 16])
                        didx = p1i.tile([P, T // 16], dt.int16, tag="didx")
                        nc.sync.dma_start(didx[:],
                                          dstidx.ap()[:, col0:col0 + T // 16])
                        # transposed dma_gather crashes the device above 512
                        # indices per call -- split into 512-index sub-calls
                        GQ = 512
                        srcg = p1.tile([P, 1, T], dt.bfloat16, tag="srcg")
                        dstg = p1.tile([P, 1, T], dt.bfloat16, tag="dstg")
                        for q in range(T // GQ):
                            qs = slice(q * GQ, (q + 1) * GQ)
                            qi = slice(q * (GQ // 16), (q + 1) * (GQ // 16))
                            nc.gpsimd.dma_gather(
                                srcg[:, :, qs], psrc[c].ap(), sidx[:, qi],
                                GQ, GQ, P, transpose=True)
                            nc.gpsimd.dma_gather(
                                dstg[:, :, qs], pdst.ap(), didx[:, qi],
                                GQ, GQ, P, transpose=True)
                        eftt = p1.tile([FE, T], dt.bfloat16, tag="eftt")
                        nc.sync.dma_start(eftt[:], eft.ap()[:, t * T:(t + 1) * T])

                        x_sb = p1.tile([P, T], dt.bfloat16, tag="x")
                        sqd = p1.tile([P, SUB], dt.bfloat16, tag="sqd")
                        for s in range(nsub):
                            sl = slice(s * SUB, (s + 1) * SUB)
                            ps = ps1.tile([P, SUB], dt.float32, tag="ps")
                            nc.tensor.matmul(ps[:], w3_sb[:], eftt[:, sl],
                                             start=True, stop=False)
                            nc.tensor.matmul(ps[:], identb[:], srcg[:, 0, sl],
                                             start=False, stop=True)
                            scol = t * nsub + s
                            nc.vector.tensor_tensor(
                                x_sb[:, sl], ps[:], dstg[:, 0, sl], Alu.add)
                            nc.vector.tensor_reduce(
                                sumc[:, scol:scol + 1], x_sb[:, sl],
                                mybir.AxisListType.X, Alu.add)
                            nc.scalar.activation(
                                sqd[:], x_sb[:, sl], Act.Square,
                                accum_out=sqc[:, scol:scol + 1])
                        blk, half = t // 2, (t % 2) * F
                        nc.scalar.dma_start(xint.ap()[blk, half:half + F, :],
                                            x_sb[0:F, :])
                        nc.scalar.dma_start(xupd.ap()[blk, half:half + F, :],
                                            x_sb[F:P, :])

            # ---------------- edge-BN stats ----------------
            sums = cp.tile([P, 2], dt.float32)
            nc.vector.tensor_reduce(sums[:, 0:1], sumc[:],
                                    mybir.AxisListType.X, Alu.add)
            nc.vector.tensor_reduce(sums[:, 1:2], sqc[:],
                                    mybir.AxisListType.X, Alu.add)
            nc.sync.dma_start(cc1i.ap(), sums[:])
            nc.gpsimd.collective_compute(
                "AllReduce", Alu.add, replica_groups=rg,
                ins=[cc1i.ap().opt()], outs=[cc1o.ap().opt()])
            gstats = cp.tile([P, 2], dt.float32)
            nc.sync.dma_start(gstats[:], cc1o.ap())

            mu = cp.tile([P, 1], dt.float32)
            nc.vector.tensor_scalar(mu[:], gstats[:, 0:1], inv_e, None, Alu.mult)
            veps = cp.tile([P, 1], dt.float32)
            # E[x^2] - mu^2 + eps  ==  (sq*inv_e - mu*mu) + eps
            musq = cp.tile([P, 1], dt.float32)
            nc.vector.tensor_tensor(musq[:], mu[:], mu[:], Alu.mult)
            nc.vector.tensor_scalar(veps[:], gstats[:, 1:2], inv_e, None,
                                    Alu.mult)
            nc.vector.tensor_tensor(veps[:], veps[:], musq[:], Alu.subtract)
            nc.vector.tensor_scalar(veps[:], veps[:], EPS, None, Alu.add)
            sdv = cp.tile([P, 1], dt.float32)
            nc.scalar.sqrt(sdv[:], veps[:])
            isd = cp.tile([P, 1], dt.float32)
            nc.vector.reciprocal(isd[:], sdv[:])
            scl = cp.tile([P, 1], dt.float32)
            nc.vector.tensor_tensor(scl[:], gv[:], isd[:], Alu.mult)
            shf = cp.tile([P, 1], dt.float32)
            nc.vector.tensor_tensor(shf[:], mu[:], scl[:], Alu.mult)
            nc.vector.tensor_tensor(shf[:], bv[:], shf[:], Alu.subtract)

            # duplicate halves: sig_* from rows 0:F, exp_* from rows F:P
            sig_s = cp.tile([P, 1], dt.float32)
            sig_b = cp.tile([P, 1], dt.float32)
            exp_s = cp.tile([P, 1], dt.float32)
            exp_b = cp.tile([P, 1], dt.float32)
            nc.vector.tensor_copy(sig_s[0:F, :], scl[0:F, :])
            nc.sync.dma_start(sig_s[F:P, :], scl[0:F, :])
            nc.vector.tensor_copy(sig_b[0:F, :], shf[0:F, :])
            nc.sync.dma_start(sig_b[F:P, :], shf[0:F, :])
            nc.sync.dma_start(exp_s[0:F, :], scl[F:P, :])
            nc.vector.tensor_copy(exp_s[F:P, :], scl[F:P, :])
            nc.sync.dma_start(exp_b[0:F, :], shf[F:P, :])
            nc.vector.tensor_copy(exp_b[F:P, :], shf[F:P, :])

            if debug:
                nc.sync.dma_start(dbg_xint.ap(), xint.ap())
                dstt = cp.tile([P, 12], dt.float32)
                nc.vector.tensor_copy(dstt[:, 0:2], sums[:])
                nc.vector.tensor_copy(dstt[:, 2:4], gstats[:])
                nc.vector.tensor_copy(dstt[:, 4:5], scl[:])
                nc.vector.tensor_copy(dstt[:, 5:6], shf[:])
                nc.vector.tensor_copy(dstt[:, 6:7], sig_s[:])
                nc.vector.tensor_copy(dstt[:, 7:8], sig_b[:])
                nc.vector.tensor_copy(dstt[:, 8:9], exp_s[:])
                nc.vector.tensor_copy(dstt[:, 9:10], exp_b[:])
                nc.sync.dma_start(dbg_st.ap(), dstt[:])

            # ---------------- pass 2 ----------------
            G = cfg["G"]
            with tc.tile_pool(name="p2g", bufs=G + 2) as p2g, \
                 tc.tile_pool(name="p2", bufs=3) as p2, \
                 tc.tile_pool(name="p2i", bufs=3) as p2i, \
                 tc.tile_pool(name="ps2", bufs=4, space="PSUM") as ps2:
                for b0 in range(0, NBLK, G):
                    blks = range(b0, min(b0 + G, NBLK))
                    gates = {}
                    for b in blks:
                        xi = p2.tile([P, T], dt.bfloat16, tag="xi")
                        nc.sync.dma_start(xi[:], xint.ap()[b, :, :])
                        gate = p2g.tile([P, T], dt.bfloat16, tag="gate")
                        nc.scalar.activation(gate[:], xi[:], Act.Sigmoid,
                                             bias=sig_b[:], scale=sig_s[:])
                        gates[b] = gate
                    for b in blks:
                        xu = p2.tile([P, T], dt.bfloat16, tag="xu")
                        nc.sync.dma_start(xu[:], xupd.ap()[b, :, :])
                        u = p2.tile([P, T], dt.float32, tag="u")
                        nc.scalar.activation(u[:], xu[:], Act.Exp,
                                             bias=exp_b[:], scale=exp_s[:])
                        sp = p2.tile([P, T], dt.float32, tag="sp")
                        nc.scalar.activation(sp[:], u[:], Act.Ln, bias=1.0,
                                             scale=1.0)
                        gate = gates.pop(b)
                        msga = p2.tile([F, T], dt.float32, tag="msga")
                        nc.vector.tensor_tensor(msga[:], gate[0:F, :],
                                                sp[0:F, :], Alu.mult)
                        msgb = p2.tile([F, T], dt.float32, tag="msgb")
                        nc.vector.tensor_tensor(msgb[:], gate[F:P, :],
                                                sp[F:P, :], Alu.mult)
                        ssrc = p2.tile([P, 2 * T // P, F], dt.float32,
                                       tag="ssrc")
                        ntr = T // P  # transposes per msg half
                        per_ps = SUB // F  # transposed [P,F] blocks per psum
                        for q in range(2 * T // (P * per_ps)):
                            pst = ps2.tile([P, SUB], dt.float32, tag="pst")
                            for k in range(per_ps):
                                j = q * per_ps + k
                                src = msga if j < ntr else msgb
                                jj = j % ntr
                                nc.tensor.transpose(
                                    pst[:, k * F:(k + 1) * F],
                                    src[:, jj * P:(jj + 1) * P],
                                    identf[:])
                            nc.vector.tensor_copy(
                                ssrc[:, q * per_ps:(q + 1) * per_ps, :],
                                pst[:])
                        didx2 = p2i.tile([P, 2 * T // 16], dt.int16,
                                         tag="didx2")
                        nc.sync.dma_start(
                            didx2[:],
                            dstidx.ap()[:, b * (2 * T // 16):
                                        (b + 1) * (2 * T // 16)])
                        off = 0
                        for r, sr in enumerate(SEGS):
                            ri = (nseg if (r == 0 and b % 2) else r)
                            nc.gpsimd.dma_scatter_add(
                                aggd[ri].ap().flatten_outer_dims(),
                                ssrc[:, off // P:(off + sr) // P, :],
                                didx2[:, off // 16:(off + sr) // 16],
                                sr, sr, F)
                            off += sr

            if debug:
                nc.sync.dma_start(dbg_agg.ap(), aggd[0].ap())

            # ---------------- phase 3 (chunked over node groups) ------
            with tc.tile_pool(name="p3", bufs=1) as p3, \
                 tc.tile_pool(name="p3c", bufs=3) as p3c, \
                 tc.tile_pool(name="p3w", bufs=2) as p3w, \
                 tc.tile_pool(name="ps3", bufs=4, space="PSUM") as ps3:
                gpt = SUB // P  # groups per psum tile / chunk
                aggT = p3.tile([F, NGRP * P], dt.float32)
                for q0 in range(0, NGRP, gpt):
                    nq = min(gpt, NGRP - q0)
                    ac = p3c.tile([P, gpt, F], dt.float32, tag="ac")
                    nc.sync.dma_start(
                        ac[:, :nq, :],
                        aggd[0].ap()[q0:q0 + nq].rearrange("g p d -> p g d"))
                    for r in range(1, nseg + 1):
                        at = p3c.tile([P, gpt, F], dt.float32, tag="at")
                        nc.sync.dma_start(
                            at[:, :nq, :],
                            aggd[r].ap()[q0:q0 + nq].rearrange("g p d -> p g d"))
                        nc.vector.tensor_tensor(ac[:, :nq, :], ac[:, :nq, :],
                                                at[:, :nq, :], Alu.add)
                    pst = ps3.tile([F, SUB], dt.float32, tag="pst3")
                    for k in range(nq):
                        nc.tensor.transpose(
                            pst[:, k * P:(k + 1) * P],
                            ac[:, k, :], identf128[:])
                    nc.vector.tensor_copy(
                        aggT[:, q0 * P:(q0 + nq) * P], pst[:, :nq * P])

                Rr = cfg["R"]
                if R_PAD > Rr:
                    # zero trash-node columns so pad values stay bounded
                    nc.vector.memset(aggT[:, Rr:], 0.0)
                nchunk3 = 8
                cb = [(Rr * i) // nchunk3 for i in range(nchunk3 + 1)]
                nsum = p3.tile([F, 2 * nchunk3], dt.float32)
                for i in range(nchunk3):
                    sl = slice(cb[i], cb[i + 1])
                    nc.vector.tensor_reduce(nsum[:, 2 * i:2 * i + 1],
                                            aggT[:, sl],
                                            mybir.AxisListType.X, Alu.add)
                    sq = p3w.tile([F, (NGRP * P) // nchunk3 + P], dt.float32,
                                  tag="sq")
                    w = cb[i + 1] - cb[i]
                    nc.vector.tensor_tensor(sq[:, :w], aggT[:, sl],
                                            aggT[:, sl], Alu.mult)
                    nc.vector.tensor_reduce(nsum[:, 2 * i + 1:2 * i + 2],
                                            sq[:, :w],
                                            mybir.AxisListType.X, Alu.add)
                nsum2 = p3.tile([F, 2], dt.float32)
                nc.vector.tensor_reduce(
                    nsum2[:, 0:1],
                    nsum[:].rearrange("p (a b) -> p b a", b=2)[:, 0, :],
                    mybir.AxisListType.X, Alu.add)
                nc.vector.tensor_reduce(
                    nsum2[:, 1:2],
                    nsum[:].rearrange("p (a b) -> p b a", b=2)[:, 1, :],
                    mybir.AxisListType.X, Alu.add)
                nsum = nsum2
                nc.sync.dma_start(cc2i.ap(), nsum[:])
                nc.gpsimd.collective_compute(
                    "AllReduce", Alu.add, replica_groups=rg,
                    ins=[cc2i.ap().opt()], outs=[cc2o.ap().opt()])
                gs2 = p3.tile([F, 2], dt.float32)
                nc.sync.dma_start(gs2[:], cc2o.ap())

                mu2 = p3.tile([F, 1], dt.float32)
                nc.vector.tensor_scalar(mu2[:], gs2[:, 0:1], inv_n, None,
                                        Alu.mult)
                ve2 = p3.tile([F, 1], dt.float32)
                ms2 = p3.tile([F, 1], dt.float32)
                nc.vector.tensor_tensor(ms2[:], mu2[:], mu2[:], Alu.mult)
                nc.vector.tensor_scalar(ve2[:], gs2[:, 1:2], inv_n, None,
                                        Alu.mult)
                nc.vector.tensor_tensor(ve2[:], ve2[:], ms2[:], Alu.subtract)
                nc.vector.tensor_scalar(ve2[:], ve2[:], EPS, None, Alu.add)
                sd2 = p3.tile([F, 1], dt.float32)
                nc.scalar.sqrt(sd2[:], ve2[:])
                is2 = p3.tile([F, 1], dt.float32)
                nc.vector.reciprocal(is2[:], sd2[:])
                sc2 = p3.tile([F, 1], dt.float32)
                nc.vector.tensor_tensor(sc2[:], gbn_sb[:], is2[:], Alu.mult)
                sh2 = p3.tile([F, 1], dt.float32)
                nc.vector.tensor_tensor(sh2[:], mu2[:], sc2[:], Alu.mult)
                nc.vector.tensor_tensor(sh2[:], bbn_sb[:], sh2[:], Alu.subtract)

                cw = ((NGRP // nchunk3) + 1) * P
                for i in range(nchunk3):
                    c0 = min(NGRP * P, i * cw)
                    c1 = min(NGRP * P, (i + 1) * cw)
                    if c1 <= c0:
                        continue
                    w = c1 - c0
                    nftc = p3w.tile([F, cw], dt.float32, tag="nftc")
                    nc.sync.dma_start(nftc[:, :w], nft.ap()[:, c0:c1])
                    s1 = p3w.tile([F, cw], dt.float32, tag="s1")
                    nc.vector.tensor_scalar(s1[:, :w], aggT[:, c0:c1],
                                            sc2[:], sh2[:], Alu.mult, Alu.add)
                    nc.vector.tensor_tensor(s1[:, :w], s1[:, :w], nftc[:, :w],
                                            Alu.add)
                    u3 = p3w.tile([F, cw], dt.float32, tag="u3")
                    nc.scalar.activation(u3[:, :w], s1[:, :w], Act.Exp)
                    o3 = p3w.tile([F, cw], dt.float32, tag="o3")
                    nc.scalar.activation(o3[:, :w], u3[:, :w], Act.Ln,
                                         bias=1.0, scale=1.0)
                    nc.sync.dma_start(outT.ap()[:, c0:c1], o3[:, :w])

    nc.compile()
    return nc


_CACHE = {}


def _prep(inputs, T=2048, g_batch=6):
    nf = np.ascontiguousarray(np.asarray(inputs["node_feats"], np.float32))
    ef = np.ascontiguousarray(np.asarray(inputs["edge_feats"], np.float32))
    src = np.asarray(inputs["src"], np.int64)
    dst = np.asarray(inputs["dst"], np.int64)
    Wi = np.asarray(inputs["W_int"], np.float32)
    Wu = np.asarray(inputs["W_upd"], np.float32)
    N, Fn = nf.shape
    E, FE = ef.shape
    assert Fn == F
    cfg = _cfg(N, E, FE, T=T, g_batch=g_batch)
    NCh, CH, R, NCc = cfg["NCHUNK"], cfg["CH"], cfg["R"], cfg["NC"]

    # b_int/b_upd are dropped: a constant bias shifts mean equally and
    # cancels inside BatchNorm.
    Psrc = (nf @ np.concatenate([Wi[:F], Wu[:F]], axis=1)).astype(BF16)
    Pdst = (nf @ np.concatenate([Wi[F:2 * F], Wu[F:2 * F]], axis=1)).astype(BF16)
    W3 = np.concatenate([Wi[2 * F:], Wu[2 * F:]], axis=1).astype(BF16)

    core = dst // R
    chunk = src // CH
    key = core * NCh + chunk
    order = np.lexsort((src, key))
    counts = np.bincount(key, minlength=NCc * NCh)
    gstart = np.zeros(NCc * NCh + 1, np.int64)
    np.cumsum(counts, out=gstart[1:])

    # ---- occurrence-rank block filling -------------------------------
    # dma_scatter_add cannot accumulate duplicate indices within one call
    # (the CCE read-modify-write races between M2S reads and S2M writes),
    # so each block of B edges is split into rank segments: seg r holds
    # the (r+1)-th occurrences of dst values within the block, each seg
    # internally dst-unique, scattered by its own call into its own agg
    # buffer. Calls on one buffer are WAW-serialized by Tile.
    B = 2 * T

    def occ_ranks(d):
        o = np.argsort(d, kind="stable")
        sd = d[o]
        newrun = np.r_[True, sd[1:] != sd[:-1]]
        ii = np.arange(len(d))
        runstart = np.maximum.accumulate(np.where(newrun, ii, 0))
        occ = np.empty(len(d), np.int64)
        occ[o] = ii - runstart
        return occ

    prof = np.zeros(256, np.float64)
    npool = 0
    for g in range(NCc * NCh):
        dd = dst[order[gstart[g]:gstart[g + 1]]]
        for p0 in range(0, len(dd), B):
            oc = occ_ranks(dd[p0:p0 + B])
            bc = np.bincount(oc, minlength=256)[:256]
            prof += bc
            npool += 1
    prof /= max(npool, 1)
    segs = []
    for r in range(1, 256):
        if prof[r] < 24:
            break
        s_r = max(128, int(round(prof[r] / 128)) * 128)
        if sum(segs) + s_r > B - 512:
            break
        segs.append(s_r)
    SEGS = [B - sum(segs)] + segs
    cfg["SEGS"] = tuple(SEGS)
    soff = np.cumsum([0] + SEGS)

    def fill_chunk(eidx):
        blocks = []
        carry = np.empty(0, np.int64)
        ptr = 0
        n = len(eidx)
        while ptr < n or len(carry):
            take = min(B - len(carry), n - ptr)
            pool = np.concatenate([carry, eidx[ptr:ptr + take]])
            ptr += take
            oc = occ_ranks(dst[pool])
            slots = np.full(B, -1, np.int64)
            used = np.zeros(len(pool), bool)
            for r, sr in enumerate(SEGS):
                cand = np.flatnonzero(oc == r)[:sr]
                slots[soff[r]:soff[r] + len(cand)] = pool[cand]
                used[cand] = True
            carry = pool[~used]
            blocks.append(slots)
        return blocks

    core_blocks = []
    nbc = 0
    for c in range(NCc):
        per_chunk = []
        for k in range(NCh):
            g = c * NCh + k
            blks = fill_chunk(order[gstart[g]:gstart[g + 1]])
            nbc = max(nbc, len(blks))
            per_chunk.append(blks)
        core_blocks.append(per_chunk)

    tpc = 2 * nbc
    KT = tpc * T
    ETOT = NCh * KT
    cfg["TPC"], cfg["ETOT"] = tpc, ETOT

    in_maps = []
    psrc_arrs = []
    for k in range(NCh):
        tab = np.zeros((CH + 1, P), BF16)
        hi = min((k + 1) * CH, N)
        tab[: hi - k * CH] = Psrc[k * CH: hi]
        psrc_arrs.append(tab)
    gvec = np.concatenate([np.asarray(inputs["g_int"], np.float32),
                           np.asarray(inputs["g_upd"], np.float32)])[:, None]
    bvec = np.concatenate([np.asarray(inputs["be_int"], np.float32),
                           np.asarray(inputs["be_upd"], np.float32)])[:, None]
    gbn = np.asarray(inputs["g_bn"], np.float32)[:, None]
    bbn = np.asarray(inputs["be_bn"], np.float32)[:, None]

    for c in range(NCc):
        src_l = np.full(ETOT, CH, np.int16)
        dst_l = np.full(ETOT, R, np.int16)
        eft = np.zeros((FE, ETOT), BF16)
        for k in range(NCh):
            slotc = np.full(KT, -1, np.int64)
            blks = core_blocks[c][k]
            for bi, blk in enumerate(blks):
                slotc[bi * B:(bi + 1) * B] = blk
            mask = slotc >= 0
            sel = slotc[mask]
            pos = np.flatnonzero(mask) + k * KT
            src_l[pos] = (src[sel] - k * CH).astype(np.int16)
            dst_l[pos] = (dst[sel] - c * R).astype(np.int16)
            eft[:, pos] = ef[sel].T
        # verify each scatter segment is dst-unique (trash pads excluded)
        for b0 in range(0, ETOT, B):
            for r in range(len(SEGS)):
                seg = dst_l[b0 + soff[r]:b0 + soff[r + 1]]
                seg = seg[seg != R]
                assert len(np.unique(seg)) == len(seg), "seg dup!"
        pd = np.zeros((cfg["R_PAD"], P), BF16)
        pd[:R] = Pdst[c * R:(c + 1) * R]
        nft = np.zeros((F, cfg["R_PAD"]), np.float32)
        nft[:, :R] = nf[c * R:(c + 1) * R].T
        m = {
            "pdst": pd,
            "eft": eft,
            "srcidx": np.ascontiguousarray(
                np.tile(src_l.reshape(ETOT // 16, 16).T, (P // 16, 1))),
            "dstidx": np.ascontiguousarray(
                np.tile(dst_l.reshape(ETOT // 16, 16).T, (P // 16, 1))),
            "nft": nft,
            "w3": W3,
            "gvec": gvec, "bvec": bvec, "gbn": gbn, "bbn": bbn,
        }
        for k in range(NCh):
            m[f"psrc{k}"] = psrc_arrs[k]
        in_maps.append(m)
    return cfg, in_maps


def _run(inputs, T=2048, g_batch=6, trace=False):
    cfg, in_maps = _prep(inputs, T=T, g_batch=g_batch)
    ck = (cfg["N"], cfg["E"], cfg["FE"], cfg["T"], cfg["TPC"],
          cfg["G"], cfg["SEGS"])
    if ck not in _CACHE:
        _CACHE[ck] = build_graph(cfg)
    nc = _CACHE[ck]
    res = run_bass_kernel_spmd(nc, in_maps, core_ids=list(range(cfg["NC"])),
                               trace=trace)
    R = cfg["R"]
    out = np.concatenate(
        [np.asarray(res.results[c]["outT"])[:, :R].T for c in range(cfg["NC"])],
        axis=0)
    return np.ascontiguousarray(out, dtype=np.float32), res


def kernel(**inputs) -> np.ndarray:
    out, _ = _run(inputs)
    return out



# revision 11
# speedup vs baseline: 1.8520x; 1.8520x over previous
"""CGCNN conv kernel for 8 TRN2 NeuronCores (Bass/Tile).

Strategy (edge-parallel, dst-sharded, scatter/gather-minimized):
  z @ W = psrc[src] + pdst[dst] + ef @ W3 with host-prefolded
  psrc = nf @ [Wi[:64]|Wu[:64]], pdst = nf @ [Wi[64:128]|Wu[64:128]].
  Edges are sorted by dst into 125-node tiles (100 per core); within a
  tile they are sorted by src-quarter (int16 gather range) then src.
  - The dst term and the final segment-sum use one-hot matrices built
    on-device (is_equal against iota) and matmuls - no dma_scatter_add
    and no dst gather at all.  Only the src term needs dma_gather
    (non-transposed, 1024-idx calls, rotated over 4 SWDGE queues).
  - Pass 1 assembles x feat-major in PSUM (W3 matmul + one-hot dst
    expansion + PE-transposed gathered src chunks), accumulates the
    per-feature sum of squares via ACT Square, spills x bf16.
  - Edge-BN means are computed exactly on host from degree counts;
    only sumsq is AllReduced ([128,1]).
  - Pass 2 reloads x; sigmoid+softplus share one Exp table
    (sigmoid = 1/(1+exp(-xs))), Ln(1+e) for softplus, batched G tiles
    per table switch; msg chunks are PE-transposed and segment-summed
    by one-hot matmul into a per-tile PSUM bank, transposed once into
    an SBUF-resident aggT [64, R_PAD].
  - Node-BN stats AllReduce [64,2]; out = softplus(nf + bn(agg))
    feat-major; host transposes back.
"""

import sys

import numpy as np

for _p in ("/opt/trn_rl_repo", "/root/.axon_site/_ro/trn_rl_repo"):
    if _p not in sys.path:
        sys.path.append(_p)

import ml_dtypes
from concourse import bacc, bass, mybir
from concourse import tile as ctile
from concourse.bass_utils import run_bass_kernel_spmd
from concourse.masks import make_identity

P = 128
F = 64
FE = 32
N = 100_000
E = 1_600_000
NC = 8
R = N // NC            # 12500 dst nodes per core
GRP = 125              # dst nodes per tile
NT = R // GRP          # 100 tiles per core
CH = 25_000            # src rows per int16 gather table (4 quarters)
NQ = 4
R_PAD = 12544          # >= 125*99+128, multiple of 128
IDX_CAP = 1024         # max indices per dma_gather call
EPS = 1e-5
BF16 = ml_dtypes.bfloat16

Alu = mybir.AluOpType
Act = mybir.ActivationFunctionType
dt = mybir.dt


def build_graph(chq, debug=False):
    """chq: [NT, NQ] int array, chunks per (tile, quarter)."""
    C_t = chq.sum(axis=1)              # chunks per tile
    ch0 = np.zeros(NT + 1, np.int64)   # global chunk offset per tile
    np.cumsum(C_t, out=ch0[1:])
    CTOT = int(ch0[-1])
    NIDX = 128 * CTOT
    GMAX = 4                           # chunks per assembly group
    inv_e = 1.0 / float(E)
    inv_n = 1.0 / float(N)

    nc = bacc.Bacc("TRN2", target_bir_lowering=False, debug=False,
                   num_devices=NC, num_swdge_queues=4)

    psrcq = [nc.dram_tensor(f"psrcq{q}", [CH + 1, P], dt.bfloat16,
                            kind="ExternalInput") for q in range(NQ)]
    pdst = nc.dram_tensor("pdst", [R_PAD, P], dt.bfloat16,
                          kind="ExternalInput")
    eft = nc.dram_tensor("eft", [FE, NIDX], dt.bfloat16,
                         kind="ExternalInput")
    srcidx = nc.dram_tensor("srcidx", [P, NIDX // 16], dt.int16,
                            kind="ExternalInput")
    dstrel = nc.dram_tensor("dstrel", [P, CTOT], dt.float32,
                            kind="ExternalInput")
    dstrel2 = nc.dram_tensor("dstrel2", [1, NIDX], dt.float32,
                             kind="ExternalInput")
    nft = nc.dram_tensor("nft", [F, R_PAD], dt.float32,
                         kind="ExternalInput")
    w3 = nc.dram_tensor("w3", [FE, P], dt.bfloat16, kind="ExternalInput")
    iotac = nc.dram_tensor("iotac", [P, 1], dt.float32,
                           kind="ExternalInput")
    iotar = nc.dram_tensor("iotar", [P, P], dt.float32,
                           kind="ExternalInput")
    mu_e = nc.dram_tensor("mu_e", [P, 1], dt.float32, kind="ExternalInput")
    gv = nc.dram_tensor("gv", [P, 1], dt.float32, kind="ExternalInput")
    bv = nc.dram_tensor("bv", [P, 1], dt.float32, kind="ExternalInput")
    gbn = nc.dram_tensor("gbn", [F, 1], dt.float32, kind="ExternalInput")
    bbn = nc.dram_tensor("bbn", [F, 1], dt.float32, kind="ExternalInput")
    outT = nc.dram_tensor("outT", [F, R_PAD], dt.float32,
                          kind="ExternalOutput")

    xsp = nc.dram_tensor("xsp", [CTOT, P, P], dt.bfloat16, kind="Internal")
    cc1i = nc.dram_tensor("cc1i", [P, 1], dt.float32, kind="Internal")
    cc1o = nc.dram_tensor("cc1o", [P, 1], dt.float32, kind="Internal",
                          addr_space="Shared")
    cc2i = nc.dram_tensor("cc2i", [F, 2], dt.float32, kind="Internal")
    cc2o = nc.dram_tensor("cc2o", [F, 2], dt.float32, kind="Internal",
                          addr_space="Shared")
    rg = [list(range(NC))]

    if debug:
        dbg_x = nc.dram_tensor("dbg_x", [CTOT, P, P], dt.bfloat16,
                               kind="ExternalOutput")
        dbg_agg = nc.dram_tensor("dbg_agg", [F, R_PAD], dt.float32,
                                 kind="ExternalOutput")
        dbg_st = nc.dram_tensor("dbg_st", [P, 8], dt.float32,
                                kind="ExternalOutput")

    # per-(tile, quarter) gather call list: (q, chunk_off_in_tile, nchunks)
    calls = []
    for t in range(NT):
        cl = []
        off = 0
        for q in range(NQ):
            left = int(chq[t, q])
            while left > 0:
                take = min(left, IDX_CAP // 128)
                cl.append((q, off, take))
                off += take
                left -= take
        calls.append(cl)

    qrot = [0]

    with ctile.TileContext(nc) as tc:
        with tc.tile_pool(name="const", bufs=1) as cp:
            w3_sb = cp.tile([FE, P], dt.bfloat16)
            nc.sync.dma_start(w3_sb[:], w3.ap())
            identb = cp.tile([P, P], dt.bfloat16)
            make_identity(nc, identb[:])
            identf = cp.tile([P, P], dt.float32)
            make_identity(nc, identf[:])
            ones1 = cp.tile([1, P], dt.float32)
            nc.vector.memset(ones1[:], 1.0)
            iotac_sb = cp.tile([P, 1], dt.float32)
            nc.sync.dma_start(iotac_sb[:], iotac.ap())
            iotar_sb = cp.tile([P, P], dt.float32)
            nc.sync.dma_start(iotar_sb[:], iotar.ap())
            mu_sb = cp.tile([P, 1], dt.float32)
            nc.sync.dma_start(mu_sb[:], mu_e.ap())
            gv_sb = cp.tile([P, 1], dt.float32)
            nc.sync.dma_start(gv_sb[:], gv.ap())
            bv_sb = cp.tile([P, 1], dt.float32)
            nc.sync.dma_start(bv_sb[:], bv.ap())
            gbn_sb = cp.tile([F, 1], dt.float32)
            nc.sync.dma_start(gbn_sb[:], gbn.ap())
            bbn_sb = cp.tile([F, 1], dt.float32)
            nc.sync.dma_start(bbn_sb[:], bbn.ap())

            ngrp_max = int(max((int(C_t[t]) + GMAX - 1) // GMAX
                               for t in range(NT)))
            sq_acc = cp.tile([P, NT * ngrp_max], dt.float32)
            nc.vector.memset(sq_acc[:], 0.0)
            aggT = cp.tile([F, R_PAD], dt.float32)

            # ---------------- pass 1 ----------------
            with tc.tile_pool(name="p1", bufs=3) as p1, \
                 tc.tile_pool(name="p1s", bufs=6) as p1s, \
                 tc.tile_pool(name="ps1", bufs=2, space="PSUM") as ps1, \
                 tc.tile_pool(name="psT", bufs=2, space="PSUM") as psT, \
                 tc.tile_pool(name="psb", bufs=2, space="PSUM") as psb:
                for t in range(NT):
                    C = int(C_t[t])
                    c0 = int(ch0[t])
                    sidx = p1s.tile([P, C * 8], dt.int16, tag="sidx")
                    nc.sync.dma_start(sidx[:],
                                      srcidx.ap()[:, c0 * 8:(c0 + C) * 8])
                    dr2 = p1s.tile([1, C * P], dt.float32, tag="dr2")
                    nc.sync.dma_start(dr2[:],
                                      dstrel2.ap()[:, c0 * P:(c0 + C) * P])
                    eft_sb = p1.tile([FE, C * P], dt.bfloat16, tag="eft")
                    nc.scalar.dma_start(eft_sb[:],
                                        eft.ap()[:, c0 * P:(c0 + C) * P])
                    pd_sb = p1s.tile([P, P], dt.bfloat16, tag="pd")
                    nc.sync.dma_start(pd_sb[:],
                                      pdst.ap()[t * GRP:t * GRP + P, :])
                    srcg = p1.tile([P, C, P], dt.bfloat16, tag="srcg")
                    for (q, coff, nch) in calls[t]:
                        nc.gpsimd.dma_gather(
                            srcg[:, coff:coff + nch, :], psrcq[q].ap(),
                            sidx[:, coff * 8:(coff + nch) * 8],
                            nch * 128, nch * 128, P, transpose=False,
                            queue_num=qrot[0] % 4)
                        qrot[0] += 1

                    x_sb = p1.tile([P, C, P], dt.bfloat16, tag="x")
                    sqs = p1s.tile([P, GMAX * P], dt.bfloat16, tag="sqs")
                    for g0 in range(0, C, GMAX):
                        ng = min(GMAX, C - g0)
                        w = ng * P
                        sl = slice(g0 * P, g0 * P + w)
                        # broadcast dstrel along partitions via matmul
                        dstb = psb.tile([P, GMAX * P], dt.float32,
                                        tag="dstb")
                        nc.tensor.matmul(dstb[:, :w], ones1[:],
                                         dr2[:, sl], start=True, stop=True)
                        s2w = p1s.tile([P, GMAX * P], dt.bfloat16,
                                       tag="s2w")
                        nc.vector.tensor_scalar(
                            s2w[:, :w], dstb[:, :w], iotac_sb[:], None,
                            Alu.is_equal)
                        psx = ps1.tile([P, GMAX * P], dt.float32, tag="psx")
                        nc.tensor.matmul(psx[:, :w], w3_sb[:],
                                         eft_sb[:, sl],
                                         start=True, stop=False)
                        nc.tensor.matmul(psx[:, :w], pd_sb[:],
                                         s2w[:, :w],
                                         start=False, stop=True)
                        sgt = psT.tile([P, GMAX * P], dt.bfloat16,
                                       tag="sgt")
                        for k in range(ng):
                            nc.tensor.matmul(
                                sgt[:, (k * P):(k + 1) * P],
                                srcg[:, g0 + k, :], identb[:],
                                is_transpose=True, start=True, stop=True)
                        sgs = p1s.tile([P, GMAX * P], dt.bfloat16,
                                       tag="sgs")
                        nc.vector.tensor_copy(sgs[:, :w], sgt[:, :w])
                        nc.vector.tensor_tensor(
                            x_sb[:, g0:g0 + ng, :], psx[:, :w],
                            sgs[:, :w], Alu.add)
                        nc.scalar.activation(
                            sqs[:, :w], x_sb[:, g0:g0 + ng, :], Act.Square,
                            accum_out=sq_acc[:, t * ngrp_max + g0 // GMAX:
                                             t * ngrp_max + g0 // GMAX + 1])
                    nc.scalar.dma_start(
                        xsp.ap()[c0:c0 + C].rearrange("c f e -> f c e"),
                        x_sb[:])

            # ---------------- edge-BN stats ----------------
            ssq = cp.tile([P, 1], dt.float32)
            nc.vector.tensor_reduce(ssq[:], sq_acc[:],
                                    mybir.AxisListType.X, Alu.add)
            nc.sync.dma_start(cc1i.ap(), ssq[:])
            nc.gpsimd.collective_compute(
                "AllReduce", Alu.add, replica_groups=rg,
                ins=[cc1i.ap().opt()], outs=[cc1o.ap().opt()])
            gsq = cp.tile([P, 1], dt.float32)
            nc.sync.dma_start(gsq[:], cc1o.ap())

            veps = cp.tile([P, 1], dt.float32)
            musq = cp.tile([P, 1], dt.float32)
            nc.vector.tensor_tensor(musq[:], mu_sb[:], mu_sb[:], Alu.mult)
            nc.vector.tensor_scalar(veps[:], gsq[:], inv_e, None, Alu.mult)
            nc.vector.tensor_tensor(veps[:], veps[:], musq[:], Alu.subtract)
            nc.vector.tensor_scalar(veps[:], veps[:], EPS, None, Alu.add)
            sdv = cp.tile([P, 1], dt.float32)
            nc.scalar.sqrt(sdv[:], veps[:])
            isd = cp.tile([P, 1], dt.float32)
            nc.vector.reciprocal(isd[:], sdv[:])
            scl = cp.tile([P, 1], dt.float32)
            nc.vector.tensor_tensor(scl[:], gv_sb[:], isd[:], Alu.mult)
            shf = cp.tile([P, 1], dt.float32)
            nc.vector.tensor_tensor(shf[:], mu_sb[:], scl[:], Alu.mult)
            nc.vector.tensor_tensor(shf[:], bv_sb[:], shf[:], Alu.subtract)
            # Exp path: int rows (0:F) use exp(-(scl x + shf)); upd rows +.
            sclE = cp.tile([P, 1], dt.float32)
            shfE = cp.tile([P, 1], dt.float32)
            nc.vector.tensor_copy(sclE[:], scl[:])
            nc.vector.tensor_copy(shfE[:], shf[:])
            nc.vector.tensor_scalar(sclE[0:F, :], scl[0:F, :], -1.0, None,
                                    Alu.mult)
            nc.vector.tensor_scalar(shfE[0:F, :], shf[0:F, :], -1.0, None,
                                    Alu.mult)

            if debug:
                nc.sync.dma_start(dbg_x.ap(), xsp.ap())
                dstt = cp.tile([P, 8], dt.float32)
                nc.vector.tensor_copy(dstt[:, 0:1], ssq[:])
                nc.vector.tensor_copy(dstt[:, 1:2], gsq[:])
                nc.vector.tensor_copy(dstt[:, 2:3], scl[:])
                nc.vector.tensor_copy(dstt[:, 3:4], shf[:])
                nc.vector.tensor_copy(dstt[:, 4:5], sclE[:])
                nc.vector.tensor_copy(dstt[:, 5:6], shfE[:])
                nc.sync.dma_start(dbg_st.ap(), dstt[:])

            nc.vector.memset(aggT[:], 0.0)

            # ---------------- pass 2 ----------------
            G = 4  # tiles per activation-table batch
            with tc.tile_pool(name="p2", bufs=G + 2) as p2, \
                 tc.tile_pool(name="p2s", bufs=2) as p2s, \
                 tc.tile_pool(name="ps2", bufs=2, space="PSUM") as ps2, \
                 tc.tile_pool(name="ps2a", bufs=2, space="PSUM") as ps2a:
                for t0 in range(0, NT, G):
                    tl = range(t0, min(t0 + G, NT))
                    xs, es = {}, {}
                    for t in tl:
                        C, c0 = int(C_t[t]), int(ch0[t])
                        x2 = p2.tile([P, C, P], dt.bfloat16, tag="x2")
                        nc.sync.dma_start(
                            x2[:],
                            xsp.ap()[c0:c0 + C].rearrange(
                                "c f e -> f c e"))
                        xs[t] = x2
                    for t in tl:
                        C = int(C_t[t])
                        e_sb = p2.tile([P, C, P], dt.bfloat16, tag="e")
                        nc.scalar.activation(e_sb[:], xs[t][:], Act.Exp,
                                             bias=shfE[:], scale=sclE[:])
                        es[t] = e_sb
                    sps = {}
                    for t in tl:
                        C = int(C_t[t])
                        sp = p2.tile([F, C, P], dt.bfloat16, tag="sp")
                        nc.scalar.activation(sp[:], es[t][F:P, :, :], Act.Ln,
                                             bias=1.0, scale=1.0)
                        sps[t] = sp
                    for t in tl:
                        C, c0 = int(C_t[t]), int(ch0[t])
                        e_sb = es.pop(t)
                        g1 = p2s.tile([F, C, P], dt.float32, tag="g1")
                        nc.vector.tensor_scalar(g1[:], e_sb[0:F, :, :], 1.0,
                                                None, Alu.add)
                        g2 = p2s.tile([F, C, P], dt.float32, tag="g2")
                        nc.vector.reciprocal(g2[:], g1[:])
                        msg = p2s.tile([F, C, P], dt.bfloat16, tag="msg")
                        nc.vector.tensor_tensor(msg[:], g2[:], sps.pop(t)[:],
                                                Alu.mult)
                        drl = p2s.tile([P, C], dt.float32, tag="drl")
                        nc.sync.dma_start(drl[:], dstrel.ap()[:, c0:c0 + C])
                        agg_ps = ps2a.tile([P, F], dt.float32, tag="agg")
                        for c in range(C):
                            s_sb = p2s.tile([P, P], dt.bfloat16, tag="s")
                            nc.vector.tensor_scalar(
                                s_sb[:], iotar_sb[:], drl[:, c:c + 1],
                                None, Alu.is_equal)
                            mt_ps = ps2.tile([P, F], dt.bfloat16, tag="mt")
                            nc.tensor.matmul(mt_ps[:],
                                             msg[:, c, :],
                                             identb[0:F, 0:F],
                                             is_transpose=True,
                                             start=True, stop=True)
                            mt_sb = p2s.tile([P, F], dt.bfloat16, tag="mtc")
                            nc.vector.tensor_copy(mt_sb[:], mt_ps[:])
                            nc.tensor.matmul(agg_ps[:], s_sb[:], mt_sb[:],
                                             start=(c == 0),
                                             stop=(c == C - 1))
                        ag_sb = p2s.tile([P, F], dt.float32, tag="ag")
                        nc.vector.tensor_copy(ag_sb[:], agg_ps[:])
                        at_ps = ps2.tile([F, P], dt.float32, tag="at")
                        nc.tensor.matmul(at_ps[:], ag_sb[:], identf[:],
                                         is_transpose=True,
                                         start=True, stop=True)
                        nc.vector.tensor_copy(
                            aggT[:, t * GRP:t * GRP + GRP],
                            at_ps[:, 0:GRP])

            if debug:
                nc.sync.dma_start(dbg_agg.ap(), aggT[:])

            # ---------------- phase 3 ----------------
            with tc.tile_pool(name="p3", bufs=2) as p3:
                nch3 = 8
                cb = [(R_PAD * i) // nch3 for i in range(nch3 + 1)]
                nsum = cp.tile([F, 2 * nch3], dt.float32)
                for i in range(nch3):
                    sl = slice(cb[i], cb[i + 1])
                    w = cb[i + 1] - cb[i]
                    nc.vector.tensor_reduce(nsum[:, 2 * i:2 * i + 1],
                                            aggT[:, sl],
                                            mybir.AxisListType.X, Alu.add)
                    sq = p3.tile([F, R_PAD // nch3 + P], dt.float32,
                                 tag="sq")
                    nc.vector.tensor_tensor(sq[:, :w], aggT[:, sl],
                                            aggT[:, sl], Alu.mult)
                    nc.vector.tensor_reduce(nsum[:, 2 * i + 1:2 * i + 2],
                                            sq[:, :w],
                                            mybir.AxisListType.X, Alu.add)
                ns2 = cp.tile([F, 2], dt.float32)
                nc.vector.tensor_reduce(
                    ns2[:, 0:1],
                    nsum[:].rearrange("p (a b) -> p b a", b=2)[:, 0, :],
                    mybir.AxisListType.X, Alu.add)
                nc.vector.tensor_reduce(
                    ns2[:, 1:2],
                    nsum[:].rearrange("p (a b) -> p b a", b=2)[:, 1, :],
                    mybir.AxisListType.X, Alu.add)
                nc.sync.dma_start(cc2i.ap(), ns2[:])
                nc.gpsimd.collective_compute(
                    "AllReduce", Alu.add, replica_groups=rg,
                    ins=[cc2i.ap().opt()], outs=[cc2o.ap().opt()])
                gs2 = cp.tile([F, 2], dt.float32)
                nc.sync.dma_start(gs2[:], cc2o.ap())

                mu2 = cp.tile([F, 1], dt.float32)
                nc.vector.tensor_scalar(mu2[:], gs2[:, 0:1], inv_n, None,
                                        Alu.mult)
                ve2 = cp.tile([F, 1], dt.float32)
                ms2 = cp.tile([F, 1], dt.float32)
                nc.vector.tensor_tensor(ms2[:], mu2[:], mu2[:], Alu.mult)
                nc.vector.tensor_scalar(ve2[:], gs2[:, 1:2], inv_n, None,
                                        Alu.mult)
                nc.vector.tensor_tensor(ve2[:], ve2[:], ms2[:], Alu.subtract)
                nc.vector.tensor_scalar(ve2[:], ve2[:], EPS, None, Alu.add)
                sd2 = cp.tile([F, 1], dt.float32)
                nc.scalar.sqrt(sd2[:], ve2[:])
                is2 = cp.tile([F, 1], dt.float32)
                nc.vector.reciprocal(is2[:], sd2[:])
                sc2 = cp.tile([F, 1], dt.float32)
                nc.vector.tensor_tensor(sc2[:], gbn_sb[:], is2[:], Alu.mult)
                sh2 = cp.tile([F, 1], dt.float32)
                nc.vector.tensor_tensor(sh2[:], mu2[:], sc2[:], Alu.mult)
                nc.vector.tensor_tensor(sh2[:], bbn_sb[:], sh2[:],
                                        Alu.subtract)

                for i in range(nch3):
                    sl = slice(cb[i], cb[i + 1])
                    w = cb[i + 1] - cb[i]
                    cw = R_PAD // nch3 + P
                    nftc = p3.tile([F, cw], dt.float32, tag="nftc")
                    nc.sync.dma_start(nftc[:, :w], nft.ap()[:, sl])
                    s1 = p3.tile([F, cw], dt.float32, tag="s1")
                    nc.vector.tensor_scalar(s1[:, :w], aggT[:, sl],
                                            sc2[:], sh2[:], Alu.mult,
                                            Alu.add)
                    nc.vector.tensor_tensor(s1[:, :w], s1[:, :w],
                                            nftc[:, :w], Alu.add)
                    u3 = p3.tile([F, cw], dt.float32, tag="u3")
                    nc.scalar.activation(u3[:, :w], s1[:, :w], Act.Exp)
                    o3 = p3.tile([F, cw], dt.float32, tag="o3")
                    nc.scalar.activation(o3[:, :w], u3[:, :w], Act.Ln,
                                         bias=1.0, scale=1.0)
                    nc.sync.dma_start(outT.ap()[:, sl], o3[:, :w])

    nc.compile()
    return nc


_CACHE = {}


def _prep(inputs):
    nf = np.ascontiguousarray(np.asarray(inputs["node_feats"], np.float32))
    ef = np.ascontiguousarray(np.asarray(inputs["edge_feats"], np.float32))
    src = np.asarray(inputs["src"], np.int64)
    dst = np.asarray(inputs["dst"], np.int64)
    Wi = np.asarray(inputs["W_int"], np.float32)
    Wu = np.asarray(inputs["W_upd"], np.float32)

    Psrc = (nf @ np.concatenate([Wi[:F], Wu[:F]], axis=1)).astype(BF16)
    Pdst = (nf @ np.concatenate([Wi[F:2 * F], Wu[F:2 * F]],
                                axis=1)).astype(BF16)
    W3 = np.concatenate([Wi[2 * F:], Wu[2 * F:]], axis=1).astype(BF16)

    # b_int/b_upd dropped: constant bias cancels inside BatchNorm.
    # exact per-feature mean of x (without bias) from degree counts
    cnt_s = np.bincount(src, minlength=N).astype(np.float64)
    cnt_d = np.bincount(dst, minlength=N).astype(np.float64)
    mu = (cnt_s @ Psrc.astype(np.float64)
          + cnt_d @ Pdst.astype(np.float64)
          + ef.sum(axis=0, dtype=np.float64) @ W3.astype(np.float64)) / E
    mu = mu.astype(np.float32)[:, None]

    # ---- edge ordering: (dst tile, src quarter, src) ------------------
    gtile = dst // GRP                       # 0..799  (800 = NC*NT)
    quarter = src // CH                      # 0..3
    gq = gtile * NQ + quarter
    order = np.lexsort((src, gq))
    gq_s = gq[order]
    cnt = np.bincount(gq_s, minlength=NC * NT * NQ)
    gstart = np.zeros(NC * NT * NQ + 1, np.int64)
    np.cumsum(cnt, out=gstart[1:])

    # uniform per-tile chunk structure = max over cores
    cntc = cnt.reshape(NC, NT, NQ)
    chq = np.maximum((cntc + 127) // 128, 1).max(axis=0)   # [NT, NQ]
    C_t = chq.sum(axis=1)
    ch0 = np.zeros(NT + 1, np.int64)
    np.cumsum(C_t, out=ch0[1:])
    CTOT = int(ch0[-1])
    NIDX = 128 * CTOT
    # chunk offset of quarter q within tile t
    qoff = np.zeros((NT, NQ), np.int64)
    qoff[:, 1:] = np.cumsum(chq, axis=1)[:, :-1]
    # slot base for every (core, tile, quarter) group: core-local!
    base = (ch0[:NT, None] + qoff) * 128     # [NT, NQ]

    # per-edge final slot position (core-local index space)
    rank = np.arange(E) - gstart[gq_s]
    tq = gq_s % NQ
    tt = (gq_s // NQ) % NT
    pos = base[tt, tq] + rank
    ecore = (gq_s // (NT * NQ))

    iotac = np.arange(P, dtype=np.float32)[:, None]
    iotar = np.tile(np.arange(P, dtype=np.float32), (P, 1))
    gvec = np.concatenate([np.asarray(inputs["g_int"], np.float32),
                           np.asarray(inputs["g_upd"], np.float32)])[:, None]
    bvec = np.concatenate([np.asarray(inputs["be_int"], np.float32),
                           np.asarray(inputs["be_upd"], np.float32)])[:, None]
    gbn = np.asarray(inputs["g_bn"], np.float32)[:, None]
    bbn = np.asarray(inputs["be_bn"], np.float32)[:, None]
    psrc_tabs = []
    for q in range(NQ):
        tab = np.zeros((CH + 1, P), BF16)
        hi = min((q + 1) * CH, N)
        tab[:hi - q * CH] = Psrc[q * CH:hi]
        psrc_tabs.append(tab)

    in_maps = []
    for c in range(NC):
        m = (ecore == c)
        sel = order[m]
        p = pos[m]
        sidx = np.full(NIDX, CH, np.int16)
        sidx[p] = (src[sel] - quarter[sel] * CH).astype(np.int16)
        drel = np.full(NIDX, -1.0, np.float32)
        drel[p] = (dst[sel] - c * R - tt[m] * GRP).astype(np.float32)
        eftp = np.zeros((FE, NIDX), BF16)
        eftp[:, p] = ef[sel].T
        pd = np.zeros((R_PAD, P), BF16)
        pd[:R] = Pdst[c * R:(c + 1) * R]
        nftc = np.zeros((F, R_PAD), np.float32)
        nftc[:, :R] = nf[c * R:(c + 1) * R].T
        in_maps.append({
            "pdst": pd,
            "eft": eftp,
            "srcidx": np.ascontiguousarray(
                np.tile(sidx.reshape(NIDX // 16, 16).T, (P // 16, 1))),
            "dstrel": np.ascontiguousarray(
                drel.reshape(CTOT, P).T),
            "dstrel2": drel[None, :],
            "nft": nftc,
            "w3": W3,
            "iotac": iotac, "iotar": iotar, "mu_e": mu,
            "gv": gvec, "bv": bvec, "gbn": gbn, "bbn": bbn,
            **{f"psrcq{q}": psrc_tabs[q] for q in range(NQ)},
        })
    return chq, in_maps


def _run(inputs, trace=False):
    chq, in_maps = _prep(inputs)
    ck = chq.tobytes()
    if ck not in _CACHE:
        _CACHE[ck] = build_graph(chq)
    nc = _CACHE[ck]
    res = run_bass_kernel_spmd(nc, in_maps, core_ids=list(range(NC)),
                               trace=trace)
    out = np.concatenate(
        [np.asarray(res.results[c]["outT"])[:, :R].T for c in range(NC)],
        axis=0)
    return np.ascontiguousarray(out, dtype=np.float32), res


def kernel(**inputs) -> np.ndarray:
    out, _ = _run(inputs)
    return out


# revision 13
# speedup vs baseline: 3.0739x; 1.6598x over previous
"""CGCNN conv kernel for 8 TRN2 NeuronCores (Bass/Tile).

Strategy (edge-parallel, dst-sharded, scatter/gather-minimized):
  z @ W = psrc[src] + pdst[dst] + ef @ W3 with host-prefolded
  psrc = nf @ [Wi[:64]|Wu[:64]], pdst = nf @ [Wi[64:128]|Wu[64:128]].
  Edges are sorted by dst into 125-node tiles (100 per core); within a
  tile they are sorted by src-quarter (int16 gather range) then src.
  - The dst term and the final segment-sum use one-hot matrices built
    on-device (is_equal against iota) and matmuls - no dma_scatter_add
    and no dst gather at all.  Only the src term needs dma_gather
    (non-transposed, 1024-idx calls, rotated over 4 SWDGE queues).
  - Pass 1 assembles x feat-major in PSUM (W3 matmul + one-hot dst
    expansion + PE-transposed gathered src chunks), accumulates the
    per-feature sum of squares via ACT Square, spills x bf16.
  - Edge-BN means are computed exactly on host from degree counts;
    only sumsq is AllReduced ([128,1]).
  - Pass 2 reloads x; sigmoid+softplus share one Exp table
    (sigmoid = 1/(1+exp(-xs))), Ln(1+e) for softplus, batched G tiles
    per table switch; msg chunks are PE-transposed and segment-summed
    by one-hot matmul into a per-tile PSUM bank, transposed once into
    an SBUF-resident aggT [64, R_PAD].
  - Node-BN stats AllReduce [64,2]; out = softplus(nf + bn(agg))
    feat-major; host transposes back.
"""

import sys

import numpy as np

for _p in ("/opt/trn_rl_repo", "/root/.axon_site/_ro/trn_rl_repo"):
    if _p not in sys.path:
        sys.path.append(_p)

import ml_dtypes
from concourse import bacc, bass, mybir
from concourse import tile as ctile
from concourse.bass_utils import run_bass_kernel_spmd
from concourse.masks import make_identity

P = 128
F = 64
FE = 32
N = 100_000
E = 1_600_000
NC = 8
R = N // NC            # 12500 dst nodes per core
GRP = 125              # dst nodes per tile
NT = R // GRP          # 100 tiles per core
CH = 25_000            # src rows per int16 gather table (4 quarters)
NQ = 4
R_PAD = 12544          # >= 125*99+128, multiple of 128
IDX_CAP = 1024         # max indices per dma_gather call
EPS = 1e-5
BF16 = ml_dtypes.bfloat16

Alu = mybir.AluOpType
Act = mybir.ActivationFunctionType
dt = mybir.dt


def build_graph(chq, debug=False):
    """chq: [NT, NQ] int array, chunks per (tile, quarter)."""
    C_t = chq.sum(axis=1)              # chunks per tile
    ch0 = np.zeros(NT + 1, np.int64)   # global chunk offset per tile
    np.cumsum(C_t, out=ch0[1:])
    CTOT = int(ch0[-1])
    NIDX = 128 * CTOT
    GMAX = 4                           # chunks per assembly group
    inv_e = 1.0 / float(E)
    inv_n = 1.0 / float(N)

    nc = bacc.Bacc("TRN2", target_bir_lowering=False, debug=False,
                   num_devices=NC, num_swdge_queues=4)

    psrcq = [nc.dram_tensor(f"psrcq{q}", [CH + 1, P], dt.bfloat16,
                            kind="ExternalInput") for q in range(NQ)]
    pdst = nc.dram_tensor("pdst", [R_PAD, P], dt.bfloat16,
                          kind="ExternalInput")
    eft = nc.dram_tensor("eft", [FE, NIDX], dt.bfloat16,
                         kind="ExternalInput")
    srcidx = nc.dram_tensor("srcidx", [P, NIDX // 16], dt.int16,
                            kind="ExternalInput")
    dstrel = nc.dram_tensor("dstrel", [P, CTOT], dt.float32,
                            kind="ExternalInput")
    dstrel2 = nc.dram_tensor("dstrel2", [1, NIDX], dt.bfloat16,
                             kind="ExternalInput")
    nft = nc.dram_tensor("nft", [F, R_PAD], dt.float32,
                         kind="ExternalInput")
    w3 = nc.dram_tensor("w3", [FE, P], dt.bfloat16, kind="ExternalInput")
    iotac = nc.dram_tensor("iotac", [P, 1], dt.float32,
                           kind="ExternalInput")
    iotar = nc.dram_tensor("iotar", [P, P], dt.bfloat16,
                           kind="ExternalInput")
    mu_e = nc.dram_tensor("mu_e", [P, 1], dt.float32, kind="ExternalInput")
    gv = nc.dram_tensor("gv", [P, 1], dt.float32, kind="ExternalInput")
    bv = nc.dram_tensor("bv", [P, 1], dt.float32, kind="ExternalInput")
    gbn = nc.dram_tensor("gbn", [F, 1], dt.float32, kind="ExternalInput")
    bbn = nc.dram_tensor("bbn", [F, 1], dt.float32, kind="ExternalInput")
    outT = nc.dram_tensor("outT", [F, R_PAD], dt.float32,
                          kind="ExternalOutput")

    xsp = nc.dram_tensor("xsp", [CTOT, P, P], dt.bfloat16, kind="Internal")
    cc1i = nc.dram_tensor("cc1i", [P, 1], dt.float32, kind="Internal")
    cc1o = nc.dram_tensor("cc1o", [P, 1], dt.float32, kind="Internal",
                          addr_space="Shared")
    cc2i = nc.dram_tensor("cc2i", [F, 2], dt.float32, kind="Internal")
    cc2o = nc.dram_tensor("cc2o", [F, 2], dt.float32, kind="Internal",
                          addr_space="Shared")
    rg = [list(range(NC))]

    if debug:
        dbg_x = nc.dram_tensor("dbg_x", [CTOT, P, P], dt.bfloat16,
                               kind="ExternalOutput")
        dbg_agg = nc.dram_tensor("dbg_agg", [F, R_PAD], dt.float32,
                                 kind="ExternalOutput")
        dbg_st = nc.dram_tensor("dbg_st", [P, 8], dt.float32,
                                kind="ExternalOutput")

    # per-(tile, quarter) gather call list: (q, chunk_off_in_tile, nchunks)
    calls = []
    for t in range(NT):
        cl = []
        off = 0
        for q in range(NQ):
            left = int(chq[t, q])
            while left > 0:
                take = min(left, IDX_CAP // 128)
                cl.append((q, off, take))
                off += take
                left -= take
        calls.append(cl)

    qrot = [0]

    with ctile.TileContext(nc) as tc:
        with tc.tile_pool(name="const", bufs=1) as cp:
            w3_sb = cp.tile([FE, P], dt.bfloat16)
            nc.sync.dma_start(w3_sb[:], w3.ap())
            identb = cp.tile([P, P], dt.bfloat16)
            make_identity(nc, identb[:])
            identf = cp.tile([P, P], dt.float32)
            make_identity(nc, identf[:])
            ones1 = cp.tile([1, P], dt.bfloat16)
            nc.vector.memset(ones1[:], 1.0)
            iotac_sb = cp.tile([P, 1], dt.float32)
            nc.sync.dma_start(iotac_sb[:], iotac.ap())
            iotar_sb = cp.tile([P, P], dt.bfloat16)
            nc.sync.dma_start(iotar_sb[:], iotar.ap())
            mu_sb = cp.tile([P, 1], dt.float32)
            nc.sync.dma_start(mu_sb[:], mu_e.ap())
            gv_sb = cp.tile([P, 1], dt.float32)
            nc.sync.dma_start(gv_sb[:], gv.ap())
            bv_sb = cp.tile([P, 1], dt.float32)
            nc.sync.dma_start(bv_sb[:], bv.ap())
            gbn_sb = cp.tile([F, 1], dt.float32)
            nc.sync.dma_start(gbn_sb[:], gbn.ap())
            bbn_sb = cp.tile([F, 1], dt.float32)
            nc.sync.dma_start(bbn_sb[:], bbn.ap())

            ngrp_max = int(max((int(C_t[t]) + GMAX - 1) // GMAX
                               for t in range(NT)))
            sq_acc = cp.tile([P, NT * ngrp_max], dt.float32)
            nc.vector.memset(sq_acc[:], 0.0)
            aggT = cp.tile([F, R_PAD], dt.float32)

            # ---------------- pass 1 ----------------
            with tc.tile_pool(name="p1", bufs=3) as p1, \
                 tc.tile_pool(name="p1s", bufs=6) as p1s, \
                 tc.tile_pool(name="ps1", bufs=2, space="PSUM") as ps1, \
                 tc.tile_pool(name="psT", bufs=2, space="PSUM") as psT, \
                 tc.tile_pool(name="psb", bufs=2, space="PSUM") as psb:
                for t in range(NT):
                    C = int(C_t[t])
                    c0 = int(ch0[t])
                    sidx = p1s.tile([P, C * 8], dt.int16, tag="sidx")
                    nc.sync.dma_start(sidx[:],
                                      srcidx.ap()[:, c0 * 8:(c0 + C) * 8])
                    dr2 = p1s.tile([1, C * P], dt.bfloat16, tag="dr2")
                    nc.sync.dma_start(dr2[:],
                                      dstrel2.ap()[:, c0 * P:(c0 + C) * P])
                    eft_sb = p1.tile([FE, C * P], dt.bfloat16, tag="eft")
                    nc.scalar.dma_start(eft_sb[:],
                                        eft.ap()[:, c0 * P:(c0 + C) * P])
                    pd_sb = p1s.tile([P, P], dt.bfloat16, tag="pd")
                    nc.sync.dma_start(pd_sb[:],
                                      pdst.ap()[t * GRP:t * GRP + P, :])
                    srcg = p1.tile([P, C, P], dt.bfloat16, tag="srcg")
                    for (q, coff, nch) in calls[t]:
                        nc.gpsimd.dma_gather(
                            srcg[:, coff:coff + nch, :], psrcq[q].ap(),
                            sidx[:, coff * 8:(coff + nch) * 8],
                            nch * 128, nch * 128, P, transpose=False,
                            queue_num=qrot[0] % 4)
                        qrot[0] += 1

                    x_sb = p1.tile([P, C, P], dt.bfloat16, tag="x")
                    sqs = p1s.tile([P, GMAX * P], dt.bfloat16, tag="sqs")
                    for g0 in range(0, C, GMAX):
                        ng = min(GMAX, C - g0)
                        w = ng * P
                        sl = slice(g0 * P, g0 * P + w)
                        # broadcast dstrel along partitions via matmul
                        dstb = psb.tile([P, GMAX * P], dt.float32,
                                        tag="dstb")
                        nc.tensor.matmul(dstb[:, :w], ones1[:],
                                         dr2[:, sl], start=True, stop=True)
                        s2w = p1s.tile([P, GMAX * P], dt.bfloat16,
                                       tag="s2w")
                        nc.vector.tensor_scalar(
                            s2w[:, :w], dstb[:, :w], iotac_sb[:], None,
                            Alu.is_equal)
                        psx = ps1.tile([P, GMAX * P], dt.float32, tag="psx")
                        nc.tensor.matmul(psx[:, :w], w3_sb[:],
                                         eft_sb[:, sl],
                                         start=True, stop=False)
                        nc.tensor.matmul(psx[:, :w], pd_sb[:],
                                         s2w[:, :w],
                                         start=False, stop=False)
                        sgt = psT.tile([P, GMAX * P], dt.bfloat16,
                                       tag="sgt")
                        for k in range(ng):
                            nc.tensor.matmul(
                                sgt[:, (k * P):(k + 1) * P],
                                srcg[:, g0 + k, :], identb[:],
                                is_transpose=True, start=True, stop=True)
                        sgs = p1s.tile([P, GMAX * P], dt.bfloat16,
                                       tag="sgs")
                        nc.vector.tensor_copy(sgs[:, :w], sgt[:, :w])
                        nc.tensor.matmul(psx[:, :w], identb[:],
                                         sgs[:, :w],
                                         start=False, stop=True)
                        nc.scalar.activation(
                            x_sb[:, g0:g0 + ng, :], psx[:, :w], Act.Copy)
                        nc.scalar.activation(
                            sqs[:, :w], psx[:, :w], Act.Square,
                            accum_out=sq_acc[:, t * ngrp_max + g0 // GMAX:
                                             t * ngrp_max + g0 // GMAX + 1])
                    nc.sync.dma_start(
                        xsp.ap()[c0:c0 + C].rearrange("c f e -> f c e"),
                        x_sb[:])

            # ---------------- edge-BN stats ----------------
            ssq = cp.tile([P, 1], dt.float32)
            nc.vector.tensor_reduce(ssq[:], sq_acc[:],
                                    mybir.AxisListType.X, Alu.add)
            nc.sync.dma_start(cc1i.ap(), ssq[:])
            nc.gpsimd.collective_compute(
                "AllReduce", Alu.add, replica_groups=rg,
                ins=[cc1i.ap().opt()], outs=[cc1o.ap().opt()])
            gsq = cp.tile([P, 1], dt.float32)
            nc.sync.dma_start(gsq[:], cc1o.ap())

            veps = cp.tile([P, 1], dt.float32)
            musq = cp.tile([P, 1], dt.float32)
            nc.vector.tensor_tensor(musq[:], mu_sb[:], mu_sb[:], Alu.mult)
            nc.vector.tensor_scalar(veps[:], gsq[:], inv_e, None, Alu.mult)
            nc.vector.tensor_tensor(veps[:], veps[:], musq[:], Alu.subtract)
            nc.vector.tensor_scalar(veps[:], veps[:], EPS, None, Alu.add)
            sdv = cp.tile([P, 1], dt.float32)
            nc.scalar.sqrt(sdv[:], veps[:])
            isd = cp.tile([P, 1], dt.float32)
            nc.vector.reciprocal(isd[:], sdv[:])
            scl = cp.tile([P, 1], dt.float32)
            nc.vector.tensor_tensor(scl[:], gv_sb[:], isd[:], Alu.mult)
            shf = cp.tile([P, 1], dt.float32)
            nc.vector.tensor_tensor(shf[:], mu_sb[:], scl[:], Alu.mult)
            nc.vector.tensor_tensor(shf[:], bv_sb[:], shf[:], Alu.subtract)

            if debug:
                nc.sync.dma_start(dbg_x.ap(), xsp.ap())
                dstt = cp.tile([P, 8], dt.float32)
                nc.vector.tensor_copy(dstt[:, 0:1], ssq[:])
                nc.vector.tensor_copy(dstt[:, 1:2], gsq[:])
                nc.vector.tensor_copy(dstt[:, 2:3], scl[:])
                nc.vector.tensor_copy(dstt[:, 3:4], shf[:])
                nc.sync.dma_start(dbg_st.ap(), dstt[:])

            nc.vector.memset(aggT[:], 0.0)

            # ---------------- pass 2 ----------------
            G = 4  # tiles per activation-table batch
            with tc.tile_pool(name="p2", bufs=G + 2) as p2, \
                 tc.tile_pool(name="p2s", bufs=2) as p2s, \
                 tc.tile_pool(name="ps2", bufs=2, space="PSUM") as ps2, \
                 tc.tile_pool(name="ps2a", bufs=2, space="PSUM") as ps2a:
                for t0 in range(0, NT, G):
                    tl = range(t0, min(t0 + G, NT))
                    xs, gts, es = {}, {}, {}
                    for t in tl:
                        C, c0 = int(C_t[t]), int(ch0[t])
                        x2 = p2.tile([P, C, P], dt.bfloat16, tag="x2")
                        nc.sync.dma_start(
                            x2[:],
                            xsp.ap()[c0:c0 + C].rearrange(
                                "c f e -> f c e"))
                        xs[t] = x2
                    for t in tl:
                        C = int(C_t[t])
                        gt = p2.tile([F, C, P], dt.bfloat16, tag="gt")
                        nc.scalar.activation(gt[:], xs[t][0:F, :, :],
                                             Act.Sigmoid,
                                             bias=shf[0:F, :],
                                             scale=scl[0:F, :])
                        gts[t] = gt
                    for t in tl:
                        C = int(C_t[t])
                        e_sb = p2.tile([F, C, P], dt.bfloat16, tag="e")
                        nc.scalar.activation(e_sb[:], xs[t][F:P, :, :],
                                             Act.Exp,
                                             bias=shf[F:P, :],
                                             scale=scl[F:P, :])
                        es[t] = e_sb
                    sps = {}
                    for t in tl:
                        C = int(C_t[t])
                        sp = p2.tile([F, C, P], dt.bfloat16, tag="sp")
                        nc.scalar.activation(sp[:], es.pop(t)[:], Act.Ln,
                                             bias=1.0, scale=1.0)
                        sps[t] = sp
                    for t in tl:
                        C, c0 = int(C_t[t]), int(ch0[t])
                        xs.pop(t)
                        msg = p2s.tile([F, C, P], dt.bfloat16, tag="msg")
                        nc.gpsimd.tensor_tensor(msg[:], gts.pop(t)[:],
                                                sps.pop(t)[:], Alu.mult)
                        drl = p2s.tile([P, C], dt.float32, tag="drl")
                        nc.sync.dma_start(drl[:], dstrel.ap()[:, c0:c0 + C])
                        agg_ps = ps2a.tile([P, F], dt.float32, tag="agg")
                        for g0 in range(0, C, GMAX):
                            ng = min(GMAX, C - g0)
                            mt_ps = ps2.tile([P, GMAX * F], dt.bfloat16,
                                             tag="mt")
                            for k in range(ng):
                                nc.tensor.matmul(
                                    mt_ps[:, k * F:(k + 1) * F],
                                    msg[:, g0 + k, :],
                                    identb[0:F, 0:F],
                                    is_transpose=True,
                                    start=True, stop=True)
                            mt_sb = p2s.tile([P, GMAX * F], dt.bfloat16,
                                             tag="mtc")
                            nc.scalar.activation(mt_sb[:, :ng * F],
                                                 mt_ps[:, :ng * F],
                                                 Act.Copy)
                            for k in range(ng):
                                c = g0 + k
                                s_sb = p2s.tile([P, P], dt.bfloat16,
                                                tag="s")
                                nc.vector.tensor_scalar(
                                    s_sb[:], iotar_sb[:], drl[:, c:c + 1],
                                    None, Alu.is_equal)
                                nc.tensor.matmul(
                                    agg_ps[:], s_sb[:],
                                    mt_sb[:, k * F:(k + 1) * F],
                                    start=(c == 0),
                                    stop=(c == C - 1))
                        ag_sb = p2s.tile([P, F], dt.float32, tag="ag")
                        nc.vector.tensor_copy(ag_sb[:], agg_ps[:])
                        at_ps = ps2.tile([F, P], dt.float32, tag="at")
                        nc.tensor.matmul(at_ps[:], ag_sb[:], identf[:],
                                         is_transpose=True,
                                         start=True, stop=True)
                        nc.vector.tensor_copy(
                            aggT[:, t * GRP:t * GRP + GRP],
                            at_ps[:, 0:GRP])

            if debug:
                nc.sync.dma_start(dbg_agg.ap(), aggT[:])

            # ---------------- phase 3 ----------------
            with tc.tile_pool(name="p3", bufs=2) as p3:
                nch3 = 8
                cb = [(R_PAD * i) // nch3 for i in range(nch3 + 1)]
                nsum = cp.tile([F, 2 * nch3], dt.float32)
                for i in range(nch3):
                    sl = slice(cb[i], cb[i + 1])
                    w = cb[i + 1] - cb[i]
                    nc.vector.tensor_reduce(nsum[:, 2 * i:2 * i + 1],
                                            aggT[:, sl],
                                            mybir.AxisListType.X, Alu.add)
                    sq = p3.tile([F, R_PAD // nch3 + P], dt.float32,
                                 tag="sq")
                    nc.vector.tensor_tensor(sq[:, :w], aggT[:, sl],
                                            aggT[:, sl], Alu.mult)
                    nc.vector.tensor_reduce(nsum[:, 2 * i + 1:2 * i + 2],
                                            sq[:, :w],
                                            mybir.AxisListType.X, Alu.add)
                ns2 = cp.tile([F, 2], dt.float32)
                nc.vector.tensor_reduce(
                    ns2[:, 0:1],
                    nsum[:].rearrange("p (a b) -> p b a", b=2)[:, 0, :],
                    mybir.AxisListType.X, Alu.add)
                nc.vector.tensor_reduce(
                    ns2[:, 1:2],
                    nsum[:].rearrange("p (a b) -> p b a", b=2)[:, 1, :],
                    mybir.AxisListType.X, Alu.add)
                nc.sync.dma_start(cc2i.ap(), ns2[:])
                nc.gpsimd.collective_compute(
                    "AllReduce", Alu.add, replica_groups=rg,
                    ins=[cc2i.ap().opt()], outs=[cc2o.ap().opt()])
                gs2 = cp.tile([F, 2], dt.float32)
                nc.sync.dma_start(gs2[:], cc2o.ap())

                mu2 = cp.tile([F, 1], dt.float32)
                nc.vector.tensor_scalar(mu2[:], gs2[:, 0:1], inv_n, None,
                                        Alu.mult)
                ve2 = cp.tile([F, 1], dt.float32)
                ms2 = cp.tile([F, 1], dt.float32)
                nc.vector.tensor_tensor(ms2[:], mu2[:], mu2[:], Alu.mult)
                nc.vector.tensor_scalar(ve2[:], gs2[:, 1:2], inv_n, None,
                                        Alu.mult)
                nc.vector.tensor_tensor(ve2[:], ve2[:], ms2[:], Alu.subtract)
                nc.vector.tensor_scalar(ve2[:], ve2[:], EPS, None, Alu.add)
                sd2 = cp.tile([F, 1], dt.float32)
                nc.scalar.sqrt(sd2[:], ve2[:])
                is2 = cp.tile([F, 1], dt.float32)
                nc.vector.reciprocal(is2[:], sd2[:])
                sc2 = cp.tile([F, 1], dt.float32)
                nc.vector.tensor_tensor(sc2[:], gbn_sb[:], is2[:], Alu.mult)
                sh2 = cp.tile([F, 1], dt.float32)
                nc.vector.tensor_tensor(sh2[:], mu2[:], sc2[:], Alu.mult)
                nc.vector.tensor_tensor(sh2[:], bbn_sb[:], sh2[:],
                                        Alu.subtract)

                for i in range(nch3):
                    sl = slice(cb[i], cb[i + 1])
                    w = cb[i + 1] - cb[i]
                    cw = R_PAD // nch3 + P
                    nftc = p3.tile([F, cw], dt.float32, tag="nftc")
                    nc.sync.dma_start(nftc[:, :w], nft.ap()[:, sl])
                    s1 = p3.tile([F, cw], dt.float32, tag="s1")
                    nc.vector.tensor_scalar(s1[:, :w], aggT[:, sl],
                                            sc2[:], sh2[:], Alu.mult,
                                            Alu.add)
                    nc.vector.tensor_tensor(s1[:, :w], s1[:, :w],
                                            nftc[:, :w], Alu.add)
                    u3 = p3.tile([F, cw], dt.float32, tag="u3")
                    nc.scalar.activation(u3[:, :w], s1[:, :w], Act.Exp)
                    o3 = p3.tile([F, cw], dt.float32, tag="o3")
                    nc.scalar.activation(o3[:, :w], u3[:, :w], Act.Ln,
                                         bias=1.0, scale=1.0)
                    nc.sync.dma_start(outT.ap()[:, sl], o3[:, :w])

    nc.compile()
    return nc


_CACHE = {}


def _prep(inputs):
    nf = np.ascontiguousarray(np.asarray(inputs["node_feats"], np.float32))
    ef = np.ascontiguousarray(np.asarray(inputs["edge_feats"], np.float32))
    src = np.asarray(inputs["src"], np.int64)
    dst = np.asarray(inputs["dst"], np.int64)
    Wi = np.asarray(inputs["W_int"], np.float32)
    Wu = np.asarray(inputs["W_upd"], np.float32)

    Psrc = (nf @ np.concatenate([Wi[:F], Wu[:F]], axis=1)).astype(BF16)
    Pdst = (nf @ np.concatenate([Wi[F:2 * F], Wu[F:2 * F]],
                                axis=1)).astype(BF16)
    W3 = np.concatenate([Wi[2 * F:], Wu[2 * F:]], axis=1).astype(BF16)

    # b_int/b_upd dropped: constant bias cancels inside BatchNorm.
    # exact per-feature mean of x (without bias) from degree counts
    cnt_s = np.bincount(src, minlength=N).astype(np.float64)
    cnt_d = np.bincount(dst, minlength=N).astype(np.float64)
    mu = (cnt_s @ Psrc.astype(np.float64)
          + cnt_d @ Pdst.astype(np.float64)
          + ef.sum(axis=0, dtype=np.float64) @ W3.astype(np.float64)) / E
    mu = mu.astype(np.float32)[:, None]

    # ---- edge ordering: (dst tile, src quarter, src) ------------------
    gtile = dst // GRP                       # 0..799  (800 = NC*NT)
    quarter = src // CH                      # 0..3
    gq = gtile * NQ + quarter
    order = np.lexsort((src, gq))
    gq_s = gq[order]
    cnt = np.bincount(gq_s, minlength=NC * NT * NQ)
    gstart = np.zeros(NC * NT * NQ + 1, np.int64)
    np.cumsum(cnt, out=gstart[1:])

    # uniform per-tile chunk structure = max over cores
    cntc = cnt.reshape(NC, NT, NQ)
    chq = np.maximum((cntc + 127) // 128, 1).max(axis=0)   # [NT, NQ]
    C_t = chq.sum(axis=1)
    ch0 = np.zeros(NT + 1, np.int64)
    np.cumsum(C_t, out=ch0[1:])
    CTOT = int(ch0[-1])
    NIDX = 128 * CTOT
    # chunk offset of quarter q within tile t
    qoff = np.zeros((NT, NQ), np.int64)
    qoff[:, 1:] = np.cumsum(chq, axis=1)[:, :-1]
    # slot base for every (core, tile, quarter) group: core-local!
    base = (ch0[:NT, None] + qoff) * 128     # [NT, NQ]

    # per-edge final slot position (core-local index space)
    rank = np.arange(E) - gstart[gq_s]
    tq = gq_s % NQ
    tt = (gq_s // NQ) % NT
    pos = base[tt, tq] + rank
    ecore = (gq_s // (NT * NQ))

    iotac = np.arange(P, dtype=np.float32)[:, None]
    iotar = np.tile(np.arange(P, dtype=np.float32),
                    (P, 1)).astype(BF16)
    gvec = np.concatenate([np.asarray(inputs["g_int"], np.float32),
                           np.asarray(inputs["g_upd"], np.float32)])[:, None]
    bvec = np.concatenate([np.asarray(inputs["be_int"], np.float32),
                           np.asarray(inputs["be_upd"], np.float32)])[:, None]
    gbn = np.asarray(inputs["g_bn"], np.float32)[:, None]
    bbn = np.asarray(inputs["be_bn"], np.float32)[:, None]
    psrc_tabs = []
    for q in range(NQ):
        tab = np.zeros((CH + 1, P), BF16)
        hi = min((q + 1) * CH, N)
        tab[:hi - q * CH] = Psrc[q * CH:hi]
        psrc_tabs.append(tab)

    in_maps = []
    for c in range(NC):
        m = (ecore == c)
        sel = order[m]
        p = pos[m]
        sidx = np.full(NIDX, CH, np.int16)
        sidx[p] = (src[sel] - quarter[sel] * CH).astype(np.int16)
        drel = np.full(NIDX, -1.0, np.float32)
        drel[p] = (dst[sel] - c * R - tt[m] * GRP).astype(np.float32)
        eftp = np.zeros((FE, NIDX), BF16)
        eftp[:, p] = ef[sel].T
        pd = np.zeros((R_PAD, P), BF16)
        pd[:R] = Pdst[c * R:(c + 1) * R]
        nftc = np.zeros((F, R_PAD), np.float32)
        nftc[:, :R] = nf[c * R:(c + 1) * R].T
        in_maps.append({
            "pdst": pd,
            "eft": eftp,
            "srcidx": np.ascontiguousarray(
                np.tile(sidx.reshape(NIDX // 16, 16).T, (P // 16, 1))),
            "dstrel": np.ascontiguousarray(
                drel.reshape(CTOT, P).T),
            "dstrel2": drel[None, :].astype(BF16),
            "nft": nftc,
            "w3": W3,
            "iotac": iotac, "iotar": iotar, "mu_e": mu,
            "gv": gvec, "bv": bvec, "gbn": gbn, "bbn": bbn,
            **{f"psrcq{q}": psrc_tabs[q] for q in range(NQ)},
        })
    return chq, in_maps


def _run(inputs, trace=False):
    chq, in_maps = _prep(inputs)
    ck = chq.tobytes()
    if ck not in _CACHE:
        _CACHE[ck] = build_graph(chq)
    nc = _CACHE[ck]
    res = run_bass_kernel_spmd(nc, in_maps, core_ids=list(range(NC)),
                               trace=trace)
    out = np.concatenate(
        [np.asarray(res.results[c]["outT"])[:, :R].T for c in range(NC)],
        axis=0)
    return np.ascontiguousarray(out, dtype=np.float32), res


def kernel(**inputs) -> np.ndarray:
    out, _ = _run(inputs)
    return out


# revision 15
# speedup vs baseline: 3.4791x; 1.1318x over previous
"""CGCNN conv kernel for 8 TRN2 NeuronCores (Bass/Tile).

Strategy (edge-parallel, dst-sharded, scatter/gather-minimized):
  z @ W = psrc[src] + pdst[dst] + ef @ W3 with host-prefolded
  psrc = nf @ [Wi[:64]|Wu[:64]], pdst = nf @ [Wi[64:128]|Wu[64:128]].
  Edges are sorted by dst into 125-node tiles (100 per core); within a
  tile they are sorted by src-quarter (int16 gather range) then src.
  - The dst term and the final segment-sum use one-hot matrices built
    on-device (is_equal against iota) and matmuls - no dma_scatter_add
    and no dst gather at all.  Only the src term needs dma_gather
    (non-transposed, 1024-idx calls, rotated over 4 SWDGE queues).
  - Pass 1 assembles x feat-major in PSUM (W3 matmul + one-hot dst
    expansion + PE-transposed gathered src chunks), accumulates the
    per-feature sum of squares via ACT Square, spills x bf16.
  - Edge-BN means are computed exactly on host from degree counts;
    only sumsq is AllReduced ([128,1]).
  - Pass 2 reloads x; sigmoid+softplus share one Exp table
    (sigmoid = 1/(1+exp(-xs))), Ln(1+e) for softplus, batched G tiles
    per table switch; msg chunks are PE-transposed and segment-summed
    by one-hot matmul into a per-tile PSUM bank, transposed once into
    an SBUF-resident aggT [64, R_PAD].
  - Node-BN stats AllReduce [64,2]; out = softplus(nf + bn(agg))
    feat-major; host transposes back.
"""

import sys

import numpy as np

for _p in ("/opt/trn_rl_repo", "/root/.axon_site/_ro/trn_rl_repo"):
    if _p not in sys.path:
        sys.path.append(_p)

import ml_dtypes
from concourse import bacc, bass, mybir
from concourse import tile as ctile
from concourse.bass_utils import run_bass_kernel_spmd
from concourse.masks import make_identity

P = 128
F = 64
FE = 32
N = 100_000
E = 1_600_000
NC = 8
R = N // NC            # 12500 dst nodes per core
GRP = 125              # dst nodes per tile
NT = R // GRP          # 100 tiles per core
CH = 25_000            # src rows per int16 gather table (4 quarters)
NQ = 4
R_PAD = 12544          # >= 125*99+128, multiple of 128
IDX_CAP = 1024         # max indices per dma_gather call
EPS = 1e-5
BF16 = ml_dtypes.bfloat16

Alu = mybir.AluOpType
Act = mybir.ActivationFunctionType
dt = mybir.dt


def build_graph(chq, debug=False):
    """chq: [NT, NQ] int array, chunks per (tile, quarter)."""
    C_t = chq.sum(axis=1)              # chunks per tile
    ch0 = np.zeros(NT + 1, np.int64)   # global chunk offset per tile
    np.cumsum(C_t, out=ch0[1:])
    CTOT = int(ch0[-1])
    NIDX = 128 * CTOT
    GMAX = 4                           # chunks per assembly group
    inv_e = 1.0 / float(E)
    inv_n = 1.0 / float(N)

    nc = bacc.Bacc("TRN2", target_bir_lowering=False, debug=False,
                   num_devices=NC, num_swdge_queues=4)

    psrcq = [nc.dram_tensor(f"psrcq{q}", [CH + 1, P], dt.bfloat16,
                            kind="ExternalInput") for q in range(NQ)]
    pdst = nc.dram_tensor("pdst", [R_PAD, P], dt.bfloat16,
                          kind="ExternalInput")
    eft = nc.dram_tensor("eft", [FE, NIDX], dt.bfloat16,
                         kind="ExternalInput")
    srcidx = nc.dram_tensor("srcidx", [P, NIDX // 16], dt.int16,
                            kind="ExternalInput")
    dstrel = nc.dram_tensor("dstrel", [P, CTOT], dt.float32,
                            kind="ExternalInput")
    dstrel2 = nc.dram_tensor("dstrel2", [1, NIDX], dt.bfloat16,
                             kind="ExternalInput")
    nft = nc.dram_tensor("nft", [F, R_PAD], dt.float32,
                         kind="ExternalInput")
    w3 = nc.dram_tensor("w3", [FE, P], dt.bfloat16, kind="ExternalInput")
    iotac = nc.dram_tensor("iotac", [P, 1], dt.float32,
                           kind="ExternalInput")
    iotar = nc.dram_tensor("iotar", [P, P], dt.bfloat16,
                           kind="ExternalInput")
    mu_e = nc.dram_tensor("mu_e", [P, 1], dt.float32, kind="ExternalInput")
    gv = nc.dram_tensor("gv", [P, 1], dt.float32, kind="ExternalInput")
    bv = nc.dram_tensor("bv", [P, 1], dt.float32, kind="ExternalInput")
    gbn = nc.dram_tensor("gbn", [F, 1], dt.float32, kind="ExternalInput")
    bbn = nc.dram_tensor("bbn", [F, 1], dt.float32, kind="ExternalInput")
    outT = nc.dram_tensor("outT", [F, R_PAD], dt.float32,
                          kind="ExternalOutput")

    xsp = nc.dram_tensor("xsp", [CTOT, P, P], dt.bfloat16, kind="Internal")
    cc1i = nc.dram_tensor("cc1i", [P, 1], dt.float32, kind="Internal")
    cc1o = nc.dram_tensor("cc1o", [P, 1], dt.float32, kind="Internal",
                          addr_space="Shared")
    cc2i = nc.dram_tensor("cc2i", [F, 2], dt.float32, kind="Internal")
    cc2o = nc.dram_tensor("cc2o", [F, 2], dt.float32, kind="Internal",
                          addr_space="Shared")
    rg = [list(range(NC))]

    if debug:
        dbg_x = nc.dram_tensor("dbg_x", [CTOT, P, P], dt.bfloat16,
                               kind="ExternalOutput")
        dbg_agg = nc.dram_tensor("dbg_agg", [F, R_PAD], dt.float32,
                                 kind="ExternalOutput")
        dbg_st = nc.dram_tensor("dbg_st", [P, 8], dt.float32,
                                kind="ExternalOutput")

    # per-(tile, quarter) gather call list: (q, chunk_off_in_tile, nchunks)
    calls = []
    for t in range(NT):
        cl = []
        off = 0
        for q in range(NQ):
            left = int(chq[t, q])
            while left > 0:
                take = min(left, IDX_CAP // 128)
                cl.append((q, off, take))
                off += take
                left -= take
        calls.append(cl)

    qrot = [0]

    with ctile.TileContext(nc) as tc:
        with tc.tile_pool(name="const", bufs=1) as cp:
            w3_sb = cp.tile([FE, P], dt.bfloat16)
            nc.sync.dma_start(w3_sb[:], w3.ap())
            identb = cp.tile([P, P], dt.bfloat16)
            make_identity(nc, identb[:])
            identf = cp.tile([P, P], dt.float32)
            make_identity(nc, identf[:])
            ones1 = cp.tile([1, P], dt.bfloat16)
            nc.vector.memset(ones1[:], 1.0)
            iotac_sb = cp.tile([P, 1], dt.float32)
            nc.sync.dma_start(iotac_sb[:], iotac.ap())
            iotar_sb = cp.tile([P, P], dt.bfloat16)
            nc.sync.dma_start(iotar_sb[:], iotar.ap())
            mu_sb = cp.tile([P, 1], dt.float32)
            nc.sync.dma_start(mu_sb[:], mu_e.ap())
            gv_sb = cp.tile([P, 1], dt.float32)
            nc.sync.dma_start(gv_sb[:], gv.ap())
            bv_sb = cp.tile([P, 1], dt.float32)
            nc.sync.dma_start(bv_sb[:], bv.ap())
            gbn_sb = cp.tile([F, 1], dt.float32)
            nc.sync.dma_start(gbn_sb[:], gbn.ap())
            bbn_sb = cp.tile([F, 1], dt.float32)
            nc.sync.dma_start(bbn_sb[:], bbn.ap())

            ngrp_max = int(max((int(C_t[t]) + GMAX - 1) // GMAX
                               for t in range(NT)))
            sq_acc = cp.tile([P, NT * ngrp_max], dt.float32)
            nc.vector.memset(sq_acc[:], 0.0)
            aggT = cp.tile([F, R_PAD], dt.float32)

            # ---------------- pass 1 ----------------
            with tc.tile_pool(name="p1", bufs=3) as p1, \
                 tc.tile_pool(name="p1s", bufs=6) as p1s, \
                 tc.tile_pool(name="ps1", bufs=2, space="PSUM") as ps1, \
                 tc.tile_pool(name="psT", bufs=2, space="PSUM") as psT, \
                 tc.tile_pool(name="psb", bufs=2, space="PSUM") as psb:
                for t in range(NT):
                    C = int(C_t[t])
                    c0 = int(ch0[t])
                    sidx = p1s.tile([P, C * 8], dt.int16, tag="sidx")
                    nc.sync.dma_start(sidx[:],
                                      srcidx.ap()[:, c0 * 8:(c0 + C) * 8])
                    dr2 = p1s.tile([1, C * P], dt.bfloat16, tag="dr2")
                    nc.sync.dma_start(dr2[:],
                                      dstrel2.ap()[:, c0 * P:(c0 + C) * P])
                    eft_sb = p1.tile([FE, C * P], dt.bfloat16, tag="eft")
                    nc.scalar.dma_start(eft_sb[:],
                                        eft.ap()[:, c0 * P:(c0 + C) * P])
                    pd_sb = p1s.tile([P, P], dt.bfloat16, tag="pd")
                    nc.sync.dma_start(pd_sb[:],
                                      pdst.ap()[t * GRP:t * GRP + P, :])
                    srcg = p1.tile([P, C, P], dt.bfloat16, tag="srcg")
                    for (q, coff, nch) in calls[t]:
                        nc.gpsimd.dma_gather(
                            srcg[:, coff:coff + nch, :], psrcq[q].ap(),
                            sidx[:, coff * 8:(coff + nch) * 8],
                            nch * 128, nch * 128, P, transpose=False,
                            queue_num=qrot[0] % 4)
                        qrot[0] += 1

                    x_sb = p1.tile([P, C, P], dt.bfloat16, tag="x")
                    sqs = p1s.tile([P, GMAX * P], dt.bfloat16, tag="sqs")
                    for g0 in range(0, C, GMAX):
                        ng = min(GMAX, C - g0)
                        w = ng * P
                        sl = slice(g0 * P, g0 * P + w)
                        # broadcast dstrel along partitions via matmul
                        dstb = psb.tile([P, GMAX * P], dt.float32,
                                        tag="dstb")
                        nc.tensor.matmul(dstb[:, :w], ones1[:],
                                         dr2[:, sl], start=True, stop=True)
                        s2w = p1s.tile([P, GMAX * P], dt.bfloat16,
                                       tag="s2w")
                        nc.vector.tensor_scalar(
                            s2w[:, :w], dstb[:, :w], iotac_sb[:], None,
                            Alu.is_equal)
                        psx = ps1.tile([P, GMAX * P], dt.float32, tag="psx")
                        nc.tensor.matmul(psx[:, :w], w3_sb[:],
                                         eft_sb[:, sl],
                                         start=True, stop=False)
                        nc.tensor.matmul(psx[:, :w], pd_sb[:],
                                         s2w[:, :w],
                                         start=False, stop=False)
                        sgt = psT.tile([P, GMAX * P], dt.bfloat16,
                                       tag="sgt")
                        for k in range(ng):
                            nc.tensor.matmul(
                                sgt[:, (k * P):(k + 1) * P],
                                srcg[:, g0 + k, :], identb[:],
                                is_transpose=True, start=True, stop=True)
                        sgs = p1s.tile([P, GMAX * P], dt.bfloat16,
                                       tag="sgs")
                        nc.vector.tensor_copy(sgs[:, :w], sgt[:, :w])
                        nc.tensor.matmul(psx[:, :w], identb[:],
                                         sgs[:, :w],
                                         start=False, stop=True)
                        nc.scalar.activation(
                            x_sb[:, g0:g0 + ng, :], psx[:, :w], Act.Copy)
                        nc.scalar.activation(
                            sqs[:, :w], psx[:, :w], Act.Square,
                            accum_out=sq_acc[:, t * ngrp_max + g0 // GMAX:
                                             t * ngrp_max + g0 // GMAX + 1])
                    nc.sync.dma_start(
                        xsp.ap()[c0:c0 + C].rearrange("c f e -> f c e"),
                        x_sb[:])

            # ---------------- edge-BN stats ----------------
            ssq = cp.tile([P, 1], dt.float32)
            nc.vector.tensor_reduce(ssq[:], sq_acc[:],
                                    mybir.AxisListType.X, Alu.add)
            nc.sync.dma_start(cc1i.ap(), ssq[:])
            nc.gpsimd.collective_compute(
                "AllReduce", Alu.add, replica_groups=rg,
                ins=[cc1i.ap().opt()], outs=[cc1o.ap().opt()])
            gsq = cp.tile([P, 1], dt.float32)
            nc.sync.dma_start(gsq[:], cc1o.ap())

            veps = cp.tile([P, 1], dt.float32)
            musq = cp.tile([P, 1], dt.float32)
            nc.vector.tensor_tensor(musq[:], mu_sb[:], mu_sb[:], Alu.mult)
            nc.vector.tensor_scalar(veps[:], gsq[:], inv_e, None, Alu.mult)
            nc.vector.tensor_tensor(veps[:], veps[:], musq[:], Alu.subtract)
            nc.vector.tensor_scalar(veps[:], veps[:], EPS, None, Alu.add)
            sdv = cp.tile([P, 1], dt.float32)
            nc.scalar.sqrt(sdv[:], veps[:])
            isd = cp.tile([P, 1], dt.float32)
            nc.vector.reciprocal(isd[:], sdv[:])
            scl = cp.tile([P, 1], dt.float32)
            nc.vector.tensor_tensor(scl[:], gv_sb[:], isd[:], Alu.mult)
            shf = cp.tile([P, 1], dt.float32)
            nc.vector.tensor_tensor(shf[:], mu_sb[:], scl[:], Alu.mult)
            nc.vector.tensor_tensor(shf[:], bv_sb[:], shf[:], Alu.subtract)

            if debug:
                nc.sync.dma_start(dbg_x.ap(), xsp.ap())
                dstt = cp.tile([P, 8], dt.float32)
                nc.vector.tensor_copy(dstt[:, 0:1], ssq[:])
                nc.vector.tensor_copy(dstt[:, 1:2], gsq[:])
                nc.vector.tensor_copy(dstt[:, 2:3], scl[:])
                nc.vector.tensor_copy(dstt[:, 3:4], shf[:])
                nc.sync.dma_start(dbg_st.ap(), dstt[:])

            nc.vector.memset(aggT[:], 0.0)

            # ---------------- pass 2 ----------------
            G = 4  # tiles per activation-table batch
            with tc.tile_pool(name="p2", bufs=G + 2) as p2, \
                 tc.tile_pool(name="p2s", bufs=2) as p2s, \
                 tc.tile_pool(name="ps2", bufs=2, space="PSUM") as ps2, \
                 tc.tile_pool(name="ps2a", bufs=2, space="PSUM") as ps2a:
                for t0 in range(0, NT, G):
                    tl = range(t0, min(t0 + G, NT))
                    xs, gts, es = {}, {}, {}
                    for t in tl:
                        C, c0 = int(C_t[t]), int(ch0[t])
                        x2 = p2.tile([P, C, P], dt.bfloat16, tag="x2")
                        nc.sync.dma_start(
                            x2[:],
                            xsp.ap()[c0:c0 + C].rearrange(
                                "c f e -> f c e"))
                        xs[t] = x2
                    for t in tl:
                        C = int(C_t[t])
                        gt = p2.tile([F, C, P], dt.bfloat16, tag="gt")
                        nc.scalar.activation(gt[:], xs[t][0:F, :, :],
                                             Act.Sigmoid,
                                             bias=shf[0:F, :],
                                             scale=scl[0:F, :])
                        gts[t] = gt
                    for t in tl:
                        C = int(C_t[t])
                        e_sb = p2.tile([F, C, P], dt.bfloat16, tag="e")
                        nc.scalar.activation(e_sb[:], xs[t][F:P, :, :],
                                             Act.Exp,
                                             bias=shf[F:P, :],
                                             scale=scl[F:P, :])
                        es[t] = e_sb
                    sps = {}
                    for t in tl:
                        C = int(C_t[t])
                        sp = p2.tile([F, C, P], dt.bfloat16, tag="sp")
                        nc.scalar.activation(sp[:], es.pop(t)[:], Act.Ln,
                                             bias=1.0, scale=1.0)
                        sps[t] = sp
                    for t in tl:
                        C, c0 = int(C_t[t]), int(ch0[t])
                        xs.pop(t)
                        msg = p2s.tile([F, C, P], dt.bfloat16, tag="msg")
                        nc.vector.tensor_tensor(msg[:], gts.pop(t)[:],
                                                sps.pop(t)[:], Alu.mult)
                        drl = p2s.tile([P, C], dt.float32, tag="drl")
                        nc.sync.dma_start(drl[:], dstrel.ap()[:, c0:c0 + C])
                        agg_ps = ps2a.tile([P, F], dt.float32, tag="agg")
                        for gi, g0 in enumerate(range(0, C, GMAX)):
                            ng = min(GMAX, C - g0)
                            mt_ps = ps2.tile([P, GMAX * F], dt.bfloat16,
                                             tag="mt")
                            for k in range(ng):
                                nc.tensor.matmul(
                                    mt_ps[:, k * F:(k + 1) * F],
                                    msg[:, g0 + k, :],
                                    identb[0:F, 0:F],
                                    is_transpose=True,
                                    start=True, stop=True)
                            mt_sb = p2s.tile([P, GMAX * F], dt.bfloat16,
                                             tag="mtc")
                            if gi % 2 == 0:
                                nc.scalar.activation(mt_sb[:, :ng * F],
                                                     mt_ps[:, :ng * F],
                                                     Act.Copy)
                            else:
                                nc.vector.tensor_copy(mt_sb[:, :ng * F],
                                                      mt_ps[:, :ng * F])
                            s4 = p2s.tile([P, GMAX, P], dt.bfloat16,
                                          tag="s4")
                            da = drl[:, g0:g0 + ng]
                            a3 = bass.AP(da.tensor, da.offset,
                                         list(da.ap) + [[0, P]])
                            ib = iotar_sb[:]
                            b3 = bass.AP(ib.tensor, ib.offset,
                                         [list(ib.ap[0]), [0, ng],
                                          list(ib.ap[1])])
                            nc.vector.tensor_tensor(s4[:, :ng, :], a3, b3,
                                                    Alu.is_equal)
                            for k in range(ng):
                                c = g0 + k
                                nc.tensor.matmul(
                                    agg_ps[:], s4[:, k, :],
                                    mt_sb[:, k * F:(k + 1) * F],
                                    start=(c == 0),
                                    stop=(c == C - 1))
                        ag_sb = p2s.tile([P, F], dt.float32, tag="ag")
                        nc.vector.tensor_copy(ag_sb[:], agg_ps[:])
                        at_ps = ps2.tile([F, P], dt.float32, tag="at")
                        nc.tensor.matmul(at_ps[:], ag_sb[:], identf[:],
                                         is_transpose=True,
                                         start=True, stop=True)
                        nc.vector.tensor_copy(
                            aggT[:, t * GRP:t * GRP + GRP],
                            at_ps[:, 0:GRP])

            if debug:
                nc.sync.dma_start(dbg_agg.ap(), aggT[:])

            # ---------------- phase 3 ----------------
            with tc.tile_pool(name="p3", bufs=2) as p3:
                nch3 = 8
                cb = [(R_PAD * i) // nch3 for i in range(nch3 + 1)]
                nsum = cp.tile([F, 2 * nch3], dt.float32)
                for i in range(nch3):
                    sl = slice(cb[i], cb[i + 1])
                    w = cb[i + 1] - cb[i]
                    nc.vector.tensor_reduce(nsum[:, 2 * i:2 * i + 1],
                                            aggT[:, sl],
                                            mybir.AxisListType.X, Alu.add)
                    sq = p3.tile([F, R_PAD // nch3 + P], dt.float32,
                                 tag="sq")
                    nc.vector.tensor_tensor(sq[:, :w], aggT[:, sl],
                                            aggT[:, sl], Alu.mult)
                    nc.vector.tensor_reduce(nsum[:, 2 * i + 1:2 * i + 2],
                                            sq[:, :w],
                                            mybir.AxisListType.X, Alu.add)
                ns2 = cp.tile([F, 2], dt.float32)
                nc.vector.tensor_reduce(
                    ns2[:, 0:1],
                    nsum[:].rearrange("p (a b) -> p b a", b=2)[:, 0, :],
                    mybir.AxisListType.X, Alu.add)
                nc.vector.tensor_reduce(
                    ns2[:, 1:2],
                    nsum[:].rearrange("p (a b) -> p b a", b=2)[:, 1, :],
                    mybir.AxisListType.X, Alu.add)
                nc.sync.dma_start(cc2i.ap(), ns2[:])
                nc.gpsimd.collective_compute(
                    "AllReduce", Alu.add, replica_groups=rg,
                    ins=[cc2i.ap().opt()], outs=[cc2o.ap().opt()])
                gs2 = cp.tile([F, 2], dt.float32)
                nc.sync.dma_start(gs2[:], cc2o.ap())

                mu2 = cp.tile([F, 1], dt.float32)
                nc.vector.tensor_scalar(mu2[:], gs2[:, 0:1], inv_n, None,
                                        Alu.mult)
                ve2 = cp.tile([F, 1], dt.float32)
                ms2 = cp.tile([F, 1], dt.float32)
                nc.vector.tensor_tensor(ms2[:], mu2[:], mu2[:], Alu.mult)
                nc.vector.tensor_scalar(ve2[:], gs2[:, 1:2], inv_n, None,
                                        Alu.mult)
                nc.vector.tensor_tensor(ve2[:], ve2[:], ms2[:], Alu.subtract)
                nc.vector.tensor_scalar(ve2[:], ve2[:], EPS, None, Alu.add)
                sd2 = cp.tile([F, 1], dt.float32)
                nc.scalar.sqrt(sd2[:], ve2[:])
                is2 = cp.tile([F, 1], dt.float32)
                nc.vector.reciprocal(is2[:], sd2[:])
                sc2 = cp.tile([F, 1], dt.float32)
                nc.vector.tensor_tensor(sc2[:], gbn_sb[:], is2[:], Alu.mult)
                sh2 = cp.tile([F, 1], dt.float32)
                nc.vector.tensor_tensor(sh2[:], mu2[:], sc2[:], Alu.mult)
                nc.vector.tensor_tensor(sh2[:], bbn_sb[:], sh2[:],
                                        Alu.subtract)

                for i in range(nch3):
                    sl = slice(cb[i], cb[i + 1])
                    w = cb[i + 1] - cb[i]
                    cw = R_PAD // nch3 + P
                    nftc = p3.tile([F, cw], dt.float32, tag="nftc")
                    nc.sync.dma_start(nftc[:, :w], nft.ap()[:, sl])
                    s1 = p3.tile([F, cw], dt.float32, tag="s1")
                    nc.vector.tensor_scalar(s1[:, :w], aggT[:, sl],
                                            sc2[:], sh2[:], Alu.mult,
                                            Alu.add)
                    nc.vector.tensor_tensor(s1[:, :w], s1[:, :w],
                                            nftc[:, :w], Alu.add)
                    u3 = p3.tile([F, cw], dt.float32, tag="u3")
                    nc.scalar.activation(u3[:, :w], s1[:, :w], Act.Exp)
                    o3 = p3.tile([F, cw], dt.float32, tag="o3")
                    nc.scalar.activation(o3[:, :w], u3[:, :w], Act.Ln,
                                         bias=1.0, scale=1.0)
                    nc.sync.dma_start(outT.ap()[:, sl], o3[:, :w])

    nc.compile()
    return nc


_CACHE = {}


def _prep(inputs):
    nf = np.ascontiguousarray(np.asarray(inputs["node_feats"], np.float32))
    ef = np.ascontiguousarray(np.asarray(inputs["edge_feats"], np.float32))
    src = np.asarray(inputs["src"], np.int64)
    dst = np.asarray(inputs["dst"], np.int64)
    Wi = np.asarray(inputs["W_int"], np.float32)
    Wu = np.asarray(inputs["W_upd"], np.float32)

    Psrc = (nf @ np.concatenate([Wi[:F], Wu[:F]], axis=1)).astype(BF16)
    Pdst = (nf @ np.concatenate([Wi[F:2 * F], Wu[F:2 * F]],
                                axis=1)).astype(BF16)
    W3 = np.concatenate([Wi[2 * F:], Wu[2 * F:]], axis=1).astype(BF16)

    # b_int/b_upd dropped: constant bias cancels inside BatchNorm.
    # exact per-feature mean of x (without bias) from degree counts
    cnt_s = np.bincount(src, minlength=N).astype(np.float64)
    cnt_d = np.bincount(dst, minlength=N).astype(np.float64)
    mu = (cnt_s @ Psrc.astype(np.float64)
          + cnt_d @ Pdst.astype(np.float64)
          + ef.sum(axis=0, dtype=np.float64) @ W3.astype(np.float64)) / E
    mu = mu.astype(np.float32)[:, None]

    # ---- edge ordering: (dst tile, src quarter, src) ------------------
    gtile = dst // GRP                       # 0..799  (800 = NC*NT)
    quarter = src // CH                      # 0..3
    gq = gtile * NQ + quarter
    order = np.lexsort((src, gq))
    gq_s = gq[order]
    cnt = np.bincount(gq_s, minlength=NC * NT * NQ)
    gstart = np.zeros(NC * NT * NQ + 1, np.int64)
    np.cumsum(cnt, out=gstart[1:])

    # uniform per-tile chunk structure = max over cores
    cntc = cnt.reshape(NC, NT, NQ)
    chq = np.maximum((cntc + 127) // 128, 1).max(axis=0)   # [NT, NQ]
    C_t = chq.sum(axis=1)
    ch0 = np.zeros(NT + 1, np.int64)
    np.cumsum(C_t, out=ch0[1:])
    CTOT = int(ch0[-1])
    NIDX = 128 * CTOT
    # chunk offset of quarter q within tile t
    qoff = np.zeros((NT, NQ), np.int64)
    qoff[:, 1:] = np.cumsum(chq, axis=1)[:, :-1]
    # slot base for every (core, tile, quarter) group: core-local!
    base = (ch0[:NT, None] + qoff) * 128     # [NT, NQ]

    # per-edge final slot position (core-local index space)
    rank = np.arange(E) - gstart[gq_s]
    tq = gq_s % NQ
    tt = (gq_s // NQ) % NT
    pos = base[tt, tq] + rank
    ecore = (gq_s // (NT * NQ))

    iotac = np.arange(P, dtype=np.float32)[:, None]
    iotar = np.tile(np.arange(P, dtype=np.float32),
                    (P, 1)).astype(BF16)
    gvec = np.concatenate([np.asarray(inputs["g_int"], np.float32),
                           np.asarray(inputs["g_upd"], np.float32)])[:, None]
    bvec = np.concatenate([np.asarray(inputs["be_int"], np.float32),
                           np.asarray(inputs["be_upd"], np.float32)])[:, None]
    gbn = np.asarray(inputs["g_bn"], np.float32)[:, None]
    bbn = np.asarray(inputs["be_bn"], np.float32)[:, None]
    psrc_tabs = []
    for q in range(NQ):
        tab = np.zeros((CH + 1, P), BF16)
        hi = min((q + 1) * CH, N)
        tab[:hi - q * CH] = Psrc[q * CH:hi]
        psrc_tabs.append(tab)

    in_maps = []
    for c in range(NC):
        m = (ecore == c)
        sel = order[m]
        p = pos[m]
        sidx = np.full(NIDX, CH, np.int16)
        sidx[p] = (src[sel] - quarter[sel] * CH).astype(np.int16)
        drel = np.full(NIDX, -1.0, np.float32)
        drel[p] = (dst[sel] - c * R - tt[m] * GRP).astype(np.float32)
        eftp = np.zeros((FE, NIDX), BF16)
        eftp[:, p] = ef[sel].T
        pd = np.zeros((R_PAD, P), BF16)
        pd[:R] = Pdst[c * R:(c + 1) * R]
        nftc = np.zeros((F, R_PAD), np.float32)
        nftc[:, :R] = nf[c * R:(c + 1) * R].T
        in_maps.append({
            "pdst": pd,
            "eft": eftp,
            "srcidx": np.ascontiguousarray(
                np.tile(sidx.reshape(NIDX // 16, 16).T, (P // 16, 1))),
            "dstrel": np.ascontiguousarray(
                drel.reshape(CTOT, P).T),
            "dstrel2": drel[None, :].astype(BF16),
            "nft": nftc,
            "w3": W3,
            "iotac": iotac, "iotar": iotar, "mu_e": mu,
            "gv": gvec, "bv": bvec, "gbn": gbn, "bbn": bbn,
            **{f"psrcq{q}": psrc_tabs[q] for q in range(NQ)},
        })
    return chq, in_maps


def _run(inputs, trace=False):
    chq, in_maps = _prep(inputs)
    ck = chq.tobytes()
    if ck not in _CACHE:
        _CACHE[ck] = build_graph(chq)
    nc = _CACHE[ck]
    res = run_bass_kernel_spmd(nc, in_maps, core_ids=list(range(NC)),
                               trace=trace)
    out = np.concatenate(
        [np.asarray(res.results[c]["outT"])[:, :R].T for c in range(NC)],
        axis=0)
    return np.ascontiguousarray(out, dtype=np.float32), res


def kernel(**inputs) -> np.ndarray:
    out, _ = _run(inputs)
    return out
